# revision 13
# baseline (speedup 1.0000x reference)
"""AttnBlockST Trainium2 kernel (fp8 DoubleRow, fused-projection version).

Two SPMD phases on 8 NeuronCores:
  phase 1 (spatial): data-parallel over b*t (32 samples -> 4/core),
    attention over hw=1024 within each (bt, c, hw) sample.
  phase 2 (temporal): data-parallel over b*h*w (2048 -> 256/core),
    attention over t=16, 8 samples packed per 128-partition group with a
    block-diagonal logit mask.

Matmuls run in fp8e4 with DoubleRow perf mode (256-wide contraction,
0.5 cyc/row).  Host-side algebra shrinks the device work:
  * logits = (Wq h + bq)^T (Wk h)  ==  h^T (M h + bz)  with
    M = Wq_eff^T Wk_eff (one projection z instead of q AND k; bk cancels
    in the softmax over keys).
  * phase 1 also fuses Wo (Wv h) = (Wo Wv) h = U h, so the attention
    output projection is applied to V up front and the O stage vanishes.
  * bv is folded into bo (bo_eff = bo + Wo bv).
  * inputs are pre-scaled by 32 (x'' = 32 x, bf16) and outputs carry the
    same 32x factor, so the final residual+bias is one
    scalar_tensor_tensor; the host divides the final output by 32.
GroupNorm statistics, softmax and accumulation stay fp32; rstd is
exp(-0.5*ln(var+eps)) so the scalar engine only ever uses the ln/exp
activation table (no table thrashing).
"""

import numpy as np
import ml_dtypes
from contextlib import ExitStack

import concourse.bass as bass
import concourse.mybir as mybir
import concourse.tile as tile
from concourse.bass_utils import run_bass_kernel_spmd

# ---- walrus workaround: split multi-wait final drain ----
from concourse.vector_clock import ScopedClock
from concourse.tile import TileContext


def _patched_drain_and_barrier(self, tick_clock, wait_clock):
    nc = self.nc
    drain_inst = nc.sync.drain()
    wait_clock.add_sem_waits(
        drain_inst.ins, ScopedClock({None: tick_clock.global_clock})
    )
    si = drain_inst.ins.sync_info
    if si is not None and len(si.on_wait) > 1:
        waits = list(si.on_wait)
        drain_inst.ins.sync_info = mybir.SyncInfo(
            on_wait=waits[:1], on_update=list(si.on_update)
        )
        for w in waits[1:]:
            n = nc.sync.nop(nofuse=True, hint="drain_wait_split")
            n.ins.sync_info = mybir.SyncInfo(on_wait=[w], on_update=[])
    nc.all_engine_barrier()
    assert self.sems is not None
    popped = nc._tile_sem_poison_stack.pop()
    assert popped is self._sem_poison
    nc.clear_and_free_semaphores(list(self.sems.allocated().values()))
    nc.all_engine_barrier()


TileContext._drain_and_barrier = _patched_drain_and_barrier

# ---- problem constants (hardcoded per spec) ----
B, C, T, H, W = 2, 512, 16, 32, 32
GROUPS = 32
EPS = 1e-6
N_CORES = 8
P = 128
CCH = C // P          # 4 channel chunks
GPC = GROUPS // CCH   # 8 groups per 128-channel chunk
GS = C // GROUPS      # 16 channels per group

L1 = H * W            # 1024 spatial positions
NS1 = (B * T) // N_CORES   # 4 samples per core, phase 1
LCH1 = L1 // P        # 8 position chunks

NT2 = 16              # temporal length
NS2 = (B * H * W) // N_CORES  # 256 samples per core, phase 2
HALF = NS2 // 2       # process in halves of 128 samples
F2 = HALF * NT2       # 2048 free columns per half
NGRP = F2 // P        # 16 groups of 8 samples per half
GB = 8                # groups per attention sub-batch

X_S = 32.0            # input/output carry scale (x'' = 32 x)
S_W = 32.0            # fp8 weight scale for wv/wo (phase 2)
S_M = 128.0           # fp8 scale for the fused M = Wq^T Wk matrix
S_OV = 128.0          # fp8 scale for the fused U = Wo Wv matrix (phase 1)
DG_S = 256.0          # diag (1/rowsum) scale into fp8 range (phase 2)
SC_EXP = float(C) ** -0.5 / S_M   # logit scale applied inside exp

F32 = mybir.dt.float32
BF16 = mybir.dt.bfloat16
F8 = mybir.dt.float8e4
AX = mybir.AxisListType.X
AF = mybir.ActivationFunctionType
DR = mybir.MatmulPerfMode.DoubleRow


def _op():
    from concourse.alu_op_type import AluOpType
    return AluOpType


def _bcast_inner(ap, n):
    """View (P, F) access pattern as (P, F, n) with stride-0 inner dim."""
    return bass.AP(tensor=ap.tensor, offset=ap.offset, ap=list(ap.ap) + [[0, n]])


def _bcast_outer(ap, n):
    """View (P, F) access pattern as (P, n, F) with stride-0 middle dim."""
    a = list(ap.ap)
    return bass.AP(tensor=ap.tensor, offset=ap.offset,
                   ap=[a[0], [0, n]] + a[1:])


def _split_waits(nc, limit=1):
    """This walrus build rejects >1 sem wait on every ISA template tested
    (LDWEIGHTS, CTRL, ACT, DVE TensorScalar); hoist extra waits onto
    same-engine NoOps placed just before."""
    ctr = [0]
    for f in nc.m.functions:
        for b in f.blocks:
            new = []
            for ins in b.instructions:
                si = getattr(ins, "sync_info", None)
                waits = list(si.on_wait) if si is not None and si.on_wait else []
                lim = limit
                if len(waits) > lim:
                    for w in waits[lim:]:
                        ctr[0] += 1
                        new.append(mybir.InstNoOp(
                            name=f"wsplit-{ctr[0]}",
                            sync_info=mybir.SyncInfo(on_wait=[w], on_update=[]),
                            bass_nofuse=True,
                            engine=ins.engine,
                        ))
                    ins.sync_info = mybir.SyncInfo(
                        on_wait=waits[:lim], on_update=list(si.on_update)
                    )
                new.append(ins)
            b.instructions = new
    return nc


# ---------------------------------------------------------------- phase 1
def build_spatial(reps=1):
    nc = bass.Bass()
    xs = nc.dram_tensor("xs", [NS1, C, L1], BF16, kind="ExternalInput")
    ys = nc.dram_tensor("ys", [NS1, C, L1], F32, kind="ExternalOutput")
    wd = {
        n: nc.dram_tensor(n, [C, C], F8, kind="ExternalInput")
        for n in ("wm", "wu")
    }
    bd = {
        n: nc.dram_tensor(n, [P, CCH], F32, kind="ExternalInput")
        for n in ("bz", "bo")
    }
    gmask_d = nc.dram_tensor("gmask", [P, GPC], F32, kind="ExternalInput")
    bmask_d = nc.dram_tensor("bmask", [GPC, P], F32, kind="ExternalInput")
    identf8_d = nc.dram_tensor("identf8", [P, P], F8, kind="ExternalInput")
    A = _op()

    with tile.TileContext(nc) as tc, ExitStack() as ctx:
        const = ctx.enter_context(tc.tile_pool(name="const", bufs=1))
        stp = ctx.enter_context(tc.tile_pool(name="stats", bufs=3))
        xp = ctx.enter_context(tc.tile_pool(name="x", bufs=2))
        hp = ctx.enter_context(tc.tile_pool(name="h", bufs=2))
        zp = ctx.enter_context(tc.tile_pool(name="z", bufs=2))
        up = ctx.enter_context(tc.tile_pool(name="u", bufs=2))
        ptp = ctx.enter_context(tc.tile_pool(name="pt", bufs=2))
        rp = ctx.enter_context(tc.tile_pool(name="r", bufs=2))
        yp = ctx.enter_context(tc.tile_pool(name="y", bufs=3))
        psA = ctx.enter_context(tc.tile_pool(name="psA", bufs=2, space="PSUM"))
        psB = ctx.enter_context(tc.tile_pool(name="psB", bufs=2, space="PSUM"))
        psS = ctx.enter_context(tc.tile_pool(name="psS", bufs=2, space="PSUM"))

        w_sb = {}
        for n in wd:
            t = const.tile([P, CCH, C], F8, tag=n)
            nc.sync.dma_start(out=t, in_=wd[n].rearrange("(k p) o -> p k o", p=P))
            w_sb[n] = t
        b_sb = {}
        for n in bd:
            t = const.tile([P, CCH], F32, tag=n)
            nc.sync.dma_start(out=t, in_=bd[n][:, :])
            b_sb[n] = t
        gmask = const.tile([P, GPC], F32, tag="gmask")
        nc.sync.dma_start(out=gmask, in_=gmask_d[:, :])
        bmask = const.tile([GPC, P], F32, tag="bmask")
        nc.sync.dma_start(out=bmask, in_=bmask_d[:, :])
        identf8 = const.tile([P, P], F8, tag="identf8")
        nc.sync.dma_start(out=identf8, in_=identf8_d[:, :])
        ones4 = const.tile([P, 2, 1], F8, tag="ones4")
        nc.vector.memset(ones4, S_OV / X_S)
        eps_t = const.tile([GPC, 1], F32, tag="eps")
        nc.vector.memset(eps_t, EPS)

        for i_rep in range(reps * NS1):
            i = i_rep % NS1
            x_sb = xp.tile([P, CCH, L1], BF16)
            nc.sync.dma_start(out=x_sb, in_=xs[i].rearrange("(k p) l -> p k l", p=P))

            # ---- GroupNorm stats (batched over chunks) -> h (fp8) ----
            mv = stp.tile([P, 2, CCH], F32, tag="mv")
            for k in range(CCH):
                xc = x_sb[:, k, :]
                st = stp.tile([P, 2, 6], F32, tag="bnst")
                nc.vector.bn_stats(out=st[:, 0, :], in_=xc[:, 0:512])
                nc.vector.bn_stats(out=st[:, 1, :], in_=xc[:, 512:1024])
                nc.vector.bn_aggr(out=mv[:, :, k], in_=st)
            me = stp.tile([P, 2, CCH], F32, tag="me")
            nc.vector.tensor_copy(out=me[:, 0, :], in_=mv[:, 0, :])
            m2 = stp.tile([P, CCH], F32, tag="m2")
            nc.vector.tensor_mul(out=m2, in0=mv[:, 0, :], in1=mv[:, 0, :])
            nc.vector.tensor_add(out=me[:, 1, :], in0=mv[:, 1, :], in1=m2)
            gs_ps = psS.tile([GPC, 2, CCH], F32, tag="st")
            nc.tensor.matmul(out=gs_ps.rearrange("g a k -> g (a k)"),
                             lhsT=gmask, rhs=me.rearrange("p a k -> p (a k)"),
                             start=True, stop=True)
            gs = stp.tile([GPC, 2, CCH], F32, tag="gs")
            nc.vector.tensor_copy(out=gs, in_=gs_ps)
            var = stp.tile([GPC, CCH], F32, tag="var")
            nc.vector.tensor_mul(out=var, in0=gs[:, 0, :], in1=gs[:, 0, :])
            var2 = stp.tile([GPC, CCH], F32, tag="var2")
            nc.vector.tensor_sub(out=var2, in0=gs[:, 1, :], in1=var)
            # rstd = exp(-0.5*ln(var+eps)) -- stays on the ln/exp table
            lnv = stp.tile([GPC, CCH], F32, tag="lnv")
            nc.scalar.activation(out=lnv, in_=var2, func=AF.Ln, bias=eps_t)
            ab = stp.tile([GPC, 2, CCH], F32, tag="ab")
            nc.scalar.activation(out=ab[:, 0, :], in_=lnv, func=AF.Exp,
                                 scale=-0.5)
            nc.vector.scalar_tensor_tensor(
                out=ab[:, 1, :], in0=gs[:, 0, :], scalar=-1.0, in1=ab[:, 0, :],
                op0=A.mult, op1=A.mult,
            )
            abc_ps = psS.tile([P, 2, CCH], F32, tag="st")
            nc.tensor.matmul(out=abc_ps.rearrange("p a k -> p (a k)"),
                             lhsT=bmask, rhs=ab.rearrange("g a k -> g (a k)"),
                             start=True, stop=True)
            abc = stp.tile([P, 2, CCH], F32, tag="abc")
            nc.vector.tensor_copy(out=abc, in_=abc_ps)
            h_sb = hp.tile([P, CCH, L1], F8, tag="h")
            for k in range(CCH):
                nc.gpsimd.tensor_scalar(
                    out=h_sb[:, k, :], in0=x_sb[:, k, :],
                    scalar1=abc[:, 0, k:k + 1], scalar2=abc[:, 1, k:k + 1],
                    op0=A.mult, op1=A.add,
                )

            # ---- z = M h + bz (c-major, fp8 DoubleRow) ----
            z_sb = zp.tile([P, CCH, L1], F8, tag="z")
            for m in range(CCH):
                ps = psA.tile([P, L1], F32, tag="mm")
                for kk in (0, 2):
                    for nb in range(2):
                        nc.tensor.matmul(
                            out=ps[:, nb * 512:(nb + 1) * 512],
                            lhsT=w_sb["wm"][:, kk:kk + 2, m * P:(m + 1) * P],
                            rhs=h_sb[:, kk:kk + 2, nb * 512:(nb + 1) * 512],
                            start=(kk == 0), stop=(kk == 2),
                            perf_mode=DR,
                        )
                nc.vector.tensor_scalar_add(
                    out=z_sb[:, m, :], in0=ps, scalar1=b_sb["bz"][:, m:m + 1])

            # ---- u^T = (Wo Wv) h, positions on partitions ----
            uT_sb = up.tile([P, LCH1, C], F8, tag="u")
            for m in range(LCH1):
                ps = psB.tile([P, C], F32, tag="u")
                for kk in (0, 2):
                    nc.tensor.matmul(
                        out=ps,
                        lhsT=h_sb[:, kk:kk + 2, m * P:(m + 1) * P],
                        rhs=w_sb["wu"][:, kk:kk + 2, :],
                        start=(kk == 0), stop=(kk == 2),
                        perf_mode=DR,
                    )
                eng = nc.scalar if (m % 2 == 0) else nc.vector
                if eng is nc.scalar:
                    eng.activation(out=uT_sb[:, m, :], in_=ps, func=AF.Copy)
                else:
                    eng.tensor_copy(out=uT_sb[:, m, :], in_=ps)

            # ---- S^T = h^T z per key chunk, exp -> p^T (fp8, direct) ----
            pt_sb = ptp.tile([P, LCH1, L1], F8, tag="ptv")
            for m in range(LCH1):
                ps_s = psA.tile([P, L1], F32, tag="mm")
                for kk in (0, 2):
                    for nb in range(2):
                        nc.tensor.matmul(
                            out=ps_s[:, nb * 512:(nb + 1) * 512],
                            lhsT=h_sb[:, kk:kk + 2, m * P:(m + 1) * P],
                            rhs=z_sb[:, kk:kk + 2, nb * 512:(nb + 1) * 512],
                            start=(kk == 0), stop=(kk == 2),
                            perf_mode=DR,
                        )
                nc.scalar.activation(out=pt_sb[:, m, :], in_=ps_s,
                                     func=AF.Exp, scale=SC_EXP)

            # ---- rowsums (per q partition-chunk) + r^T = p^T' U ----
            rs_ps = psS.tile([P, LCH1], F32, tag="rs")
            rT_sb = rp.tile([P, LCH1, C], F8, tag="rT")
            for qc in range(LCH1):
                ps_r = psB.tile([P, C], F32, tag="u")
                for j in range(4):
                    lhsT = pt_sb[:, 2 * j:2 * j + 2, qc * P:(qc + 1) * P]
                    nc.tensor.matmul(
                        out=ps_r, lhsT=lhsT,
                        rhs=uT_sb[:, 2 * j:2 * j + 2, :],
                        start=(j == 0), stop=(j == 3), perf_mode=DR,
                    )
                    nc.tensor.matmul(
                        out=rs_ps[:, qc:qc + 1], lhsT=lhsT, rhs=ones4,
                        start=(j == 0), stop=(j == 3), perf_mode=DR,
                    )
                rc = stp.tile([P, 1], F32, tag="rc")
                nc.vector.reciprocal(out=rc, in_=rs_ps[:, qc:qc + 1])
                nc.vector.tensor_scalar_mul(
                    out=rT_sb[:, qc, :], in0=ps_r, scalar1=rc)

            # ---- transpose r^T -> r, add bias + residual -> ys ----
            for m in range(CCH):
                ps_y = psA.tile([P, L1], F32, tag="mm")
                for qc in range(LCH1):
                    nc.tensor.matmul(
                        out=ps_y[:, qc * P:(qc + 1) * P],
                        lhsT=rT_sb[:, qc, m * P:(m + 1) * P], rhs=identf8,
                        start=True, stop=True,
                    )
                y_sb = yp.tile([P, L1], F32, tag="y")
                nc.vector.scalar_tensor_tensor(
                    out=y_sb, in0=ps_y, scalar=b_sb["bo"][:, m:m + 1],
                    in1=x_sb[:, m, :], op0=A.add, op1=A.add,
                )
                nc.sync.dma_start(out=ys[i, m * P:(m + 1) * P, :], in_=y_sb)
    return nc


# ---------------------------------------------------------------- phase 2
def build_temporal(reps=1):
    nc = bass.Bass()
    xt = nc.dram_tensor("xt", [C, NS2 * NT2], BF16, kind="ExternalInput")
    yt = nc.dram_tensor("yt", [C, NS2 * NT2], F32, kind="ExternalOutput")
    wd = {
        n: nc.dram_tensor(n, [C, C], F8, kind="ExternalInput")
        for n in ("wm", "wv", "wo")
    }
    bd = {
        n: nc.dram_tensor(n, [P, CCH], F32, kind="ExternalInput")
        for n in ("bz", "bo")
    }
    gmask_d = nc.dram_tensor("gmask", [P, GPC], BF16, kind="ExternalInput")
    bmask_d = nc.dram_tensor("bmask", [GPC, P], BF16, kind="ExternalInput")
    ident_d = nc.dram_tensor("ident", [P, P], BF16, kind="ExternalInput")
    blkmask_d = nc.dram_tensor("blkmask", [P, P], F32, kind="ExternalInput")
    A = _op()
    NN = HALF  # samples per half

    with tile.TileContext(nc) as tc, ExitStack() as ctx:
        const = ctx.enter_context(tc.tile_pool(name="const", bufs=1))
        stp = ctx.enter_context(tc.tile_pool(name="stats", bufs=2))
        xp = ctx.enter_context(tc.tile_pool(name="x", bufs=2))
        sqp = ctx.enter_context(tc.tile_pool(name="sq", bufs=2))
        trp = ctx.enter_context(tc.tile_pool(name="tr", bufs=2))
        tmpp = ctx.enter_context(tc.tile_pool(name="tmp", bufs=2))
        hp = ctx.enter_context(tc.tile_pool(name="h", bufs=2))
        zp = ctx.enter_context(tc.tile_pool(name="z", bufs=2))
        vp = ctx.enter_context(tc.tile_pool(name="v", bufs=2))
        pp = ctx.enter_context(tc.tile_pool(name="pm", bufs=2))
        yp = ctx.enter_context(tc.tile_pool(name="y", bufs=2))
        psA = ctx.enter_context(tc.tile_pool(name="psA", bufs=2, space="PSUM"))
        psB = ctx.enter_context(tc.tile_pool(name="psB", bufs=2, space="PSUM"))

        w_sb = {}
        for n in wd:
            t = const.tile([P, CCH, C], F8, tag=n)
            nc.sync.dma_start(out=t, in_=wd[n].rearrange("(k p) o -> p k o", p=P))
            w_sb[n] = t
        b_sb = {}
        for n in bd:
            t = const.tile([P, CCH], F32, tag=n)
            nc.sync.dma_start(out=t, in_=bd[n][:, :])
            b_sb[n] = t
        gmask = const.tile([P, GPC], BF16, tag="gmask")
        nc.sync.dma_start(out=gmask, in_=gmask_d[:, :])
        bmask = const.tile([GPC, P], BF16, tag="bmask")
        nc.sync.dma_start(out=bmask, in_=bmask_d[:, :])
        ident = const.tile([P, P], BF16, tag="ident")
        nc.sync.dma_start(out=ident, in_=ident_d[:, :])
        ident256 = const.tile([P, P], BF16, tag="ident256")
        nc.vector.tensor_scalar_mul(out=ident256, in0=ident, scalar1=DG_S)
        blkmask = const.tile([P, P], F32, tag="blkmask")
        nc.sync.dma_start(out=blkmask, in_=blkmask_d[:, :])
        eps_t = const.tile([GPC, 1], F32, tag="eps")
        nc.vector.memset(eps_t, EPS)

        xr = xt.rearrange("(k p) f -> p k f", p=P)
        yr = yt.rearrange("(k p) f -> p k f", p=P)

        for ih_rep in range(reps * 2):
            ih = ih_rep % 2
            f0 = ih * F2
            x_sb = xp.tile([P, CCH, F2], BF16)
            nc.sync.dma_start(out=x_sb, in_=xr[:, :, f0:f0 + F2])

            # ---- GroupNorm stats via halving trees ----
            sq = sqp.tile([P, CCH, F2], BF16, tag="sq")
            nc.vector.tensor_mul(
                out=sq.rearrange("p k f -> p (k f)"),
                in0=x_sb.rearrange("p k f -> p (k f)"),
                in1=x_sb.rearrange("p k f -> p (k f)"))
            me_bf = stp.tile([P, 2, CCH, NN], BF16, tag="mebf")
            with nc.allow_low_precision("GN stats tree in bf16"):
                for src_i, src in ((0, x_sb), (1, sq)):
                    v16 = src.rearrange("p k (n t) -> p (k n) t", t=NT2)
                    t8 = trp.tile([P, CCH * NN, 8], BF16, tag="t8")
                    nc.vector.tensor_tensor(
                        out=t8, in0=v16[:, :, 0:8], in1=v16[:, :, 8:16],
                        op=A.add)
                    t4 = trp.tile([P, CCH * NN, 4], BF16, tag="t4")
                    nc.vector.tensor_tensor(
                        out=t4, in0=t8[:, :, 0:4], in1=t8[:, :, 4:8],
                        op=A.add)
                    nc.vector.reduce_sum(
                        out=me_bf[:, src_i].rearrange("p k n -> p (k n)"),
                        in_=t4, axis=AX)
            gs_ps = psB.tile([GPC, 2, CCH, NN], F32, tag="sps")
            for hb in range(2):
                nc.tensor.matmul(
                    out=gs_ps.rearrange("g a k n -> g (a k n)")[:, hb * 512:(hb + 1) * 512],
                    lhsT=gmask,
                    rhs=me_bf.rearrange("p a k n -> p (a k n)")[:, hb * 512:(hb + 1) * 512],
                    start=True, stop=True,
                )
            gs = stp.tile([GPC, 2, CCH, NN], F32, tag="gs2")
            nc.vector.tensor_copy(out=gs, in_=gs_ps)
            var = stp.tile([GPC, CCH, NN], F32, tag="var2a")
            nc.vector.tensor_mul(
                out=var, in0=gs[:, 0, :, :], in1=gs[:, 0, :, :])
            var2 = stp.tile([GPC, CCH, NN], F32, tag="var2b")
            nc.vector.tensor_sub(out=var2, in0=gs[:, 1, :, :], in1=var)
            lnv = stp.tile([GPC, CCH, NN], F32, tag="lnv")
            nc.scalar.activation(
                out=lnv, in_=var2.rearrange("g k n -> g (k n)"),
                func=AF.Ln, bias=eps_t)
            ab = stp.tile([GPC, 2, CCH, NN], BF16, tag="ab2")
            nc.scalar.activation(
                out=ab[:, 0, :, :], in_=lnv, func=AF.Exp, scale=-0.5)
            nc.vector.scalar_tensor_tensor(
                out=ab[:, 1, :, :], in0=gs[:, 0, :, :], scalar=-1.0,
                in1=ab[:, 0, :, :], op0=A.mult, op1=A.mult,
            )
            abc_ps = psB.tile([P, 2, CCH, NN], F32, tag="sps")
            for hb in range(2):
                nc.tensor.matmul(
                    out=abc_ps.rearrange("p a k n -> p (a k n)")[:, hb * 512:(hb + 1) * 512],
                    lhsT=bmask,
                    rhs=ab.rearrange("g a k n -> g (a k n)")[:, hb * 512:(hb + 1) * 512],
                    start=True, stop=True,
                )
            abc = stp.tile([P, 2, CCH, NN], BF16, tag="abc2")
            nc.vector.tensor_copy(out=abc, in_=abc_ps)

            # ---- GN apply -> h (fp8) ----
            h_sb = hp.tile([P, CCH, F2], F8, tag="h")
            for k in range(CCH):
                xc3 = x_sb[:, k, :].rearrange("p (n t) -> p n t", t=NT2)
                tmp = tmpp.tile([P, F2], BF16, tag="tmp")
                nc.vector.tensor_tensor(
                    out=tmp.rearrange("p (n t) -> p n t", t=NT2),
                    in0=xc3, in1=_bcast_inner(abc[:, 0, k, :], NT2), op=A.mult,
                )
                nc.gpsimd.tensor_tensor(
                    out=h_sb[:, k, :].rearrange("p (n t) -> p n t", t=NT2),
                    in0=tmp.rearrange("p (n t) -> p n t", t=NT2),
                    in1=_bcast_inner(abc[:, 1, k, :], NT2), op=A.add,
                )

            # ---- z = M h + bz (fp8 DoubleRow) ----
            z_sb = zp.tile([P, CCH, F2], F8, tag="z")
            for m in range(CCH):
                for nbb in range(2):
                    ps = psA.tile([P, 1024], F32, tag="mm")
                    for kk in (0, 2):
                        for nb in range(2):
                            nc.tensor.matmul(
                                out=ps[:, nb * 512:(nb + 1) * 512],
                                lhsT=w_sb["wm"][:, kk:kk + 2, m * P:(m + 1) * P],
                                rhs=h_sb[:, kk:kk + 2,
                                         nbb * 1024 + nb * 512:
                                         nbb * 1024 + (nb + 1) * 512],
                                start=(kk == 0), stop=(kk == 2),
                                perf_mode=DR,
                            )
                    sl = slice(nbb * 1024, (nbb + 1) * 1024)
                    nc.vector.tensor_scalar_add(
                        out=z_sb[:, m, sl], in0=ps,
                        scalar1=b_sb["bz"][:, m:m + 1],
                    )

            # ---- v^T (fp8 DoubleRow), 2 groups per psum tile ----
            vT_sb = vp.tile([P, NGRP, C], F8, tag="v")
            for mg in range(NGRP // 2):
                ps = psA.tile([P, 1024], F32, tag="mm")
                for sub in range(2):
                    m = mg * 2 + sub
                    for kk in (0, 2):
                        nc.tensor.matmul(
                            out=ps[:, sub * 512:(sub + 1) * 512],
                            lhsT=h_sb[:, kk:kk + 2, m * P:(m + 1) * P],
                            rhs=w_sb["wv"][:, kk:kk + 2, :],
                            start=(kk == 0), stop=(kk == 2),
                            perf_mode=DR,
                        )
                nc.scalar.activation(
                    out=vT_sb[:, mg * 2:mg * 2 + 2, :],
                    in_=ps.rearrange("p (s c) -> p s c", c=C), func=AF.Copy)

            # ---- attention, GB-group batches ----
            o_sb = hp.tile([P, CCH, F2], F8, tag="h")
            for bat in range(NGRP // GB):
                g0 = bat * GB
                ps_s = psB.tile([P, GB * P], F32, tag="sps")
                for g in range(GB):
                    c0 = (g0 + g) * P
                    for kk in (0, 2):
                        nc.tensor.matmul(
                            out=ps_s[:, g * P:(g + 1) * P],
                            lhsT=z_sb[:, kk:kk + 2, c0:c0 + P],
                            rhs=h_sb[:, kk:kk + 2, c0:c0 + P],
                            start=(kk == 0), stop=(kk == 2),
                            perf_mode=DR,
                        )
                nc.vector.tensor_tensor(
                    out=ps_s.rearrange("p (g q) -> p g q", q=P),
                    in0=ps_s.rearrange("p (g q) -> p g q", q=P),
                    in1=_bcast_outer(blkmask, GB), op=A.add,
                )
                p_sb = pp.tile([P, GB * P], BF16, tag="pv")
                nc.scalar.activation(out=p_sb, in_=ps_s, func=AF.Exp,
                                     scale=SC_EXP)
                rs = stp.tile([P, GB], F32, tag="rs")
                nc.vector.reduce_sum(
                    out=rs, in_=p_sb.rearrange("p (g q) -> p g q", q=P), axis=AX)
                rc = stp.tile([P, GB], F32, tag="rc")
                nc.vector.reciprocal(out=rc, in_=rs)
                dg = stp.tile([P, GB, P], BF16, tag="dg")
                nc.gpsimd.tensor_tensor(
                    out=dg, in0=_bcast_inner(rc, P),
                    in1=_bcast_outer(ident256, GB), op=A.mult,
                )
                ps_t = psB.tile([P, GB * P], F32, tag="sps")
                for g in range(GB):
                    nc.tensor.matmul(
                        out=ps_t[:, g * P:(g + 1) * P],
                        lhsT=p_sb[:, g * P:(g + 1) * P], rhs=dg[:, g, :],
                        start=True, stop=True,
                    )
                pt_sb = pp.tile([P, GB * P], F8, tag="ptv")
                nc.vector.tensor_copy(out=pt_sb, in_=ps_t)
                for m in range(CCH):
                    ps_o = psA.tile([P, GB * P], F32, tag="mm")
                    for g in range(GB):
                        nc.tensor.matmul(
                            out=ps_o[:, g * P:(g + 1) * P],
                            lhsT=vT_sb[:, g0 + g, m * P:(m + 1) * P],
                            rhs=pt_sb[:, g * P:(g + 1) * P],
                            start=True, stop=True,
                        )
                    nc.scalar.activation(
                        out=o_sb[:, m, g0 * P:(g0 + GB) * P], in_=ps_o,
                        func=AF.Copy, scale=1.0 / (S_W * DG_S))

                # ---- r = Wo O + bo'' + x'' for this batch's columns ----
                for m in range(CCH):
                    ps_r = psA.tile([P, GB * P], F32, tag="mm")
                    for kk in (0, 2):
                        for nb in range(2):
                            nc.tensor.matmul(
                                out=ps_r[:, nb * 512:(nb + 1) * 512],
                                lhsT=w_sb["wo"][:, kk:kk + 2, m * P:(m + 1) * P],
                                rhs=o_sb[:, kk:kk + 2,
                                         g0 * P + nb * 512:
                                         g0 * P + (nb + 1) * 512],
                                start=(kk == 0), stop=(kk == 2),
                                perf_mode=DR,
                            )
                    y_sb = yp.tile([P, GB * P], F32, tag="y")
                    nc.vector.scalar_tensor_tensor(
                        out=y_sb, in0=ps_r, scalar=b_sb["bo"][:, m:m + 1],
                        in1=x_sb[:, m, g0 * P:(g0 + GB) * P],
                        op0=A.add, op1=A.add,
                    )
                    nc.sync.dma_start(
                        out=yr[:, m, f0 + g0 * P:f0 + (g0 + GB) * P], in_=y_sb)
    return nc


# ---------------------------------------------------------------- host side
F8NP = mybir.dt.np(mybir.dt.float8e4)


def _gn_fold(w, b, gamma, beta):
    """Fold GN affine into conv: W @ (hn*gamma+beta) + b."""
    w = np.asarray(w, np.float32)
    b = np.asarray(b, np.float32)
    w_eff = w * np.asarray(gamma, np.float32)[None, :]
    b_eff = b + w @ np.asarray(beta, np.float32)
    return w_eff, b_eff


def _to_f8(w, scale):
    return np.clip(np.ascontiguousarray(w) * scale, -240., 240.).astype(F8NP)


def _to_bias(b):
    return np.ascontiguousarray(np.asarray(b, np.float32).reshape(CCH, P).T)


def _consts():
    gmask1 = np.zeros((P, GPC), np.float32)
    for p in range(P):
        gmask1[p, p // GS] = 1.0 / (GS * 1)  # spatial: /16 (channel avg of means)
    gmask2 = np.zeros((P, GPC), np.float32)
    for p in range(P):
        gmask2[p, p // GS] = 1.0 / (GS * NT2)  # temporal: /256 (full group sum)
    bmask = np.zeros((GPC, P), np.float32)
    for p in range(P):
        bmask[p // GS, p] = 1.0
    ident = np.eye(P).astype(ml_dtypes.bfloat16)
    identf8 = np.eye(P).astype(F8NP)
    blk = np.full((P, P), -1e9, np.float32)
    for n in range(P // NT2):
        blk[n * NT2:(n + 1) * NT2, n * NT2:(n + 1) * NT2] = 0.0
    return gmask1, gmask2, bmask, ident, identf8, blk


_CACHE = {}


def _phase_weights(wq, bq, wk, bk, wv, bv, wo, bo, gamma, beta, phase):
    wq_eff, bq_eff = _gn_fold(wq, bq, gamma, beta)
    wk_eff, _ = _gn_fold(wk, bk, gamma, beta)   # bk cancels in softmax
    wv_eff, bv_eff = _gn_fold(wv, bv, gamma, beta)
    wo = np.asarray(wo, np.float32)
    bo = np.asarray(bo, np.float32)
    # fused logit matrix: z = M h + bz,  logits = h^T z
    m_lhsT = wq_eff.T @ wk_eff          # (c_in, c_out)
    bz = wk_eff.T @ bq_eff
    bo_eff = bo + wo @ bv_eff
    out = {
        "wm": _to_f8(m_lhsT, S_M),
        "bz": _to_bias(bz * S_M),
        "bo": _to_bias(bo_eff * X_S),
    }
    if phase == 1:
        # fused output-projection: r = (Wo Wv) h P_hat^T
        out["wu"] = _to_f8(wv_eff.T @ wo.T, S_OV)
    else:
        out["wv"] = _to_f8(wv_eff.T, S_W)
        out["wo"] = _to_f8(wo.T, S_W)
    return out


def kernel(**inputs):
    x = np.asarray(inputs["x"], np.float32)
    gmask1, gmask2, bmask, ident, identf8, blk = _consts()

    w1 = _phase_weights(inputs["wq_s"], inputs["bq_s"], inputs["wk_s"],
                        inputs["bk_s"], inputs["wv_s"], inputs["bv_s"],
                        inputs["wo_s"], inputs["bo_s"],
                        inputs["gamma_s"], inputs["beta_s"], 1)
    w2 = _phase_weights(inputs["wq_t"], inputs["bq_t"], inputs["wk_t"],
                        inputs["bk_t"], inputs["wv_t"], inputs["bv_t"],
                        inputs["wo_t"], inputs["bo_t"],
                        inputs["gamma_t"], inputs["beta_t"], 2)

    if "nc1" not in _CACHE:
        _CACHE["nc1"] = _split_waits(build_spatial())
        _CACHE["nc2"] = _split_waits(build_temporal())
    nc1, nc2 = _CACHE["nc1"], _CACHE["nc2"]

    # ---- phase 1: spatial over (b t); input pre-scaled by 32 ----
    xs = np.ascontiguousarray(
        x.transpose(0, 2, 1, 3, 4).reshape(B * T, C, L1)
    ) * X_S
    common1 = dict(gmask=gmask1, bmask=bmask, identf8=identf8, **w1)
    in_maps1 = [
        dict(xs=np.ascontiguousarray(xs[i * NS1:(i + 1) * NS1])
             .astype(ml_dtypes.bfloat16), **common1)
        for i in range(N_CORES)
    ]
    _CACHE["in_maps1"] = in_maps1
    r1 = run_bass_kernel_spmd(nc1, in_maps1, core_ids=list(range(N_CORES)),
                              **_CACHE.get("run_kwargs", {}))
    ys = np.concatenate([r1.results[i]["ys"] for i in range(N_CORES)], axis=0)
    _CACHE["last_r1"] = r1

    # ---- phase 2: temporal over (b h w); ys already carries the 32x ----
    x2 = ys.reshape(B, T, C, H, W).transpose(0, 3, 4, 2, 1)  # (b,h,w,c,t)
    x2 = x2.reshape(B * H * W, C, NT2)
    common2 = dict(gmask=gmask2.astype(ml_dtypes.bfloat16),
                   bmask=bmask.astype(ml_dtypes.bfloat16),
                   ident=ident, blkmask=blk, **w2)
    in_maps2 = []
    for i in range(N_CORES):
        shard = x2[i * NS2:(i + 1) * NS2]          # (256, 512, 16)
        xt = np.ascontiguousarray(shard.transpose(1, 0, 2)).reshape(
            C, NS2 * NT2).astype(ml_dtypes.bfloat16)
        in_maps2.append(dict(xt=xt, **common2))
    _CACHE["in_maps2"] = in_maps2
    r2 = run_bass_kernel_spmd(nc2, in_maps2, core_ids=list(range(N_CORES)),
                              **_CACHE.get("run_kwargs", {}))
    _CACHE["last_r2"] = r2

    out = np.empty((B * H * W, C, NT2), np.float32)
    for i in range(N_CORES):
        yt = r2.results[i]["yt"].reshape(C, NS2, NT2)
        out[i * NS2:(i + 1) * NS2] = yt.transpose(1, 0, 2)
    out = out.reshape(B, H, W, C, NT2).transpose(0, 3, 4, 1, 2) / X_S
    return np.ascontiguousarray(out)


# revision 22
# speedup vs baseline: 2.0511x; 2.0511x over previous
"""AttnBlockST Trainium2 kernel (fp8 DoubleRow, fused-projection version).

Two SPMD phases on 8 NeuronCores:
  phase 1 (spatial): data-parallel over b*t (32 samples -> 4/core),
    attention over hw=1024 within each (bt, c, hw) sample.
  phase 2 (temporal): data-parallel over b*h*w (2048 -> 256/core),
    attention over t=16, 8 samples packed per 128-partition group with a
    block-diagonal logit mask.

Matmuls run in fp8e4 with DoubleRow perf mode (256-wide contraction,
0.5 cyc/row).  Host-side algebra shrinks the device work:
  * logits = (Wq h + bq)^T (Wk h)  ==  h^T (M h + bz)  with
    M = Wq_eff^T Wk_eff (one projection z instead of q AND k; bk cancels
    in the softmax over keys).
  * phase 1 also fuses Wo (Wv h) = (Wo Wv) h = U h, so the attention
    output projection is applied to V up front and the O stage vanishes.
  * bv is folded into bo (bo_eff = bo + Wo bv).
  * inputs are pre-scaled by 32 (x'' = 32 x, bf16) and outputs carry the
    same 32x factor, so the final residual+bias is one
    scalar_tensor_tensor; the host divides the final output by 32.
GroupNorm statistics, softmax and accumulation stay fp32; rstd is
exp(-0.5*ln(var+eps)) so the scalar engine only ever uses the ln/exp
activation table (no table thrashing).
"""

import numpy as np
import ml_dtypes
from contextlib import ExitStack

import concourse.bass as bass
import concourse.mybir as mybir
import concourse.tile as tile
from concourse.bass_utils import run_bass_kernel_spmd

# ---- walrus workaround: split multi-wait final drain ----
from concourse.vector_clock import ScopedClock
from concourse.tile import TileContext


def _patched_drain_and_barrier(self, tick_clock, wait_clock):
    nc = self.nc
    drain_inst = nc.sync.drain()
    wait_clock.add_sem_waits(
        drain_inst.ins, ScopedClock({None: tick_clock.global_clock})
    )
    si = drain_inst.ins.sync_info
    if si is not None and len(si.on_wait) > 1:
        waits = list(si.on_wait)
        drain_inst.ins.sync_info = mybir.SyncInfo(
            on_wait=waits[:1], on_update=list(si.on_update)
        )
        for w in waits[1:]:
            n = nc.sync.nop(nofuse=True, hint="drain_wait_split")
            n.ins.sync_info = mybir.SyncInfo(on_wait=[w], on_update=[])
    nc.all_engine_barrier()
    assert self.sems is not None
    popped = nc._tile_sem_poison_stack.pop()
    assert popped is self._sem_poison
    nc.clear_and_free_semaphores(list(self.sems.allocated().values()))
    nc.all_engine_barrier()


TileContext._drain_and_barrier = _patched_drain_and_barrier

# ---- problem constants (hardcoded per spec) ----
B, C, T, H, W = 2, 512, 16, 32, 32
GROUPS = 32
EPS = 1e-6
N_CORES = 8
P = 128
CCH = C // P          # 4 channel chunks
GPC = GROUPS // CCH   # 8 groups per 128-channel chunk
GS = C // GROUPS      # 16 channels per group

L1 = H * W            # 1024 spatial positions
NS1 = (B * T) // N_CORES   # 4 samples per core, phase 1
LCH1 = L1 // P        # 8 position chunks

NT2 = 16              # temporal length
NS2 = (B * H * W) // N_CORES  # 256 samples per core, phase 2
HALF = NS2 // 4       # process in quarters of 64 samples
F2 = HALF * NT2       # 1024 free columns per quarter
NGRP = F2 // P        # 8 groups of 8 samples per quarter
GB = 4                # groups per attention sub-batch

X_S = 32.0            # input/output carry scale (x'' = 32 x)
S_W = 32.0            # fp8 weight scale for wv/wo (phase 2)
S_M = 128.0           # fp8 scale for the fused M = Wq^T Wk matrix
S_OV = 128.0          # fp8 scale for the fused U = Wo Wv matrix (phase 1)
DG_S = 256.0          # diag (1/rowsum) scale into fp8 range (phase 2)
SC_EXP = float(C) ** -0.5 / S_M   # logit scale applied inside exp

F32 = mybir.dt.float32
BF16 = mybir.dt.bfloat16
F8 = mybir.dt.float8e4
AX = mybir.AxisListType.X
AF = mybir.ActivationFunctionType
DR = mybir.MatmulPerfMode.DoubleRow


def _op():
    from concourse.alu_op_type import AluOpType
    return AluOpType


def _bcast_inner(ap, n):
    """View (P, F) access pattern as (P, F, n) with stride-0 inner dim."""
    return bass.AP(tensor=ap.tensor, offset=ap.offset, ap=list(ap.ap) + [[0, n]])


def _bcast_outer(ap, n):
    """View (P, F) access pattern as (P, n, F) with stride-0 middle dim."""
    a = list(ap.ap)
    return bass.AP(tensor=ap.tensor, offset=ap.offset,
                   ap=[a[0], [0, n]] + a[1:])


def _split_waits(nc, limit=1):
    """This walrus build rejects >1 sem wait on every ISA template tested
    (LDWEIGHTS, CTRL, ACT, DVE TensorScalar); hoist extra waits onto
    same-engine NoOps placed just before."""
    ctr = [0]
    for f in nc.m.functions:
        for b in f.blocks:
            new = []
            for ins in b.instructions:
                si = getattr(ins, "sync_info", None)
                waits = list(si.on_wait) if si is not None and si.on_wait else []
                lim = limit
                if len(waits) > lim:
                    for w in waits[lim:]:
                        ctr[0] += 1
                        new.append(mybir.InstNoOp(
                            name=f"wsplit-{ctr[0]}",
                            sync_info=mybir.SyncInfo(on_wait=[w], on_update=[]),
                            bass_nofuse=True,
                            engine=ins.engine,
                        ))
                    ins.sync_info = mybir.SyncInfo(
                        on_wait=waits[:lim], on_update=list(si.on_update)
                    )
                new.append(ins)
            b.instructions = new
    return nc


# ---------------------------------------------------------------- phase 1
def build_spatial(reps=1):
    nc = bass.Bass()
    xs = nc.dram_tensor("xs", [NS1, C, L1], BF16, kind="ExternalInput")
    ys = nc.dram_tensor("ys", [NS1, C, L1], F32, kind="ExternalOutput")
    wd = {
        n: nc.dram_tensor(n, [C, C], F8, kind="ExternalInput")
        for n in ("wm", "wu")
    }
    bd = {
        n: nc.dram_tensor(n, [P, CCH], F32, kind="ExternalInput")
        for n in ("bz", "bo")
    }
    gmask_d = nc.dram_tensor("gmask", [P, GPC], F32, kind="ExternalInput")
    bmask_d = nc.dram_tensor("bmask", [GPC, P], F32, kind="ExternalInput")
    identf8_d = nc.dram_tensor("identf8", [P, P], F8, kind="ExternalInput")
    A = _op()

    with tile.TileContext(nc) as tc, ExitStack() as ctx:
        const = ctx.enter_context(tc.tile_pool(name="const", bufs=1))
        stp = ctx.enter_context(tc.tile_pool(name="stats", bufs=3))
        xp = ctx.enter_context(tc.tile_pool(name="x", bufs=2))
        hp = ctx.enter_context(tc.tile_pool(name="h", bufs=3))
        zp = ctx.enter_context(tc.tile_pool(name="z", bufs=3))
        up = ctx.enter_context(tc.tile_pool(name="u", bufs=3))
        ptp = ctx.enter_context(tc.tile_pool(name="pt", bufs=3))
        rp = ctx.enter_context(tc.tile_pool(name="r", bufs=3))
        yp = ctx.enter_context(tc.tile_pool(name="y", bufs=3))
        psA = ctx.enter_context(tc.tile_pool(name="psA", bufs=2, space="PSUM"))
        psB = ctx.enter_context(tc.tile_pool(name="psB", bufs=2, space="PSUM"))
        psS = ctx.enter_context(tc.tile_pool(name="psS", bufs=1, space="PSUM"))

        w_sb = {}
        for n in wd:
            t = const.tile([P, CCH, C], F8, tag=n)
            nc.sync.dma_start(out=t, in_=wd[n].rearrange("(k p) o -> p k o", p=P))
            w_sb[n] = t
        b_sb = {}
        for n in bd:
            t = const.tile([P, CCH], F32, tag=n)
            nc.sync.dma_start(out=t, in_=bd[n][:, :])
            b_sb[n] = t
        gmask = const.tile([P, GPC], F32, tag="gmask")
        nc.sync.dma_start(out=gmask, in_=gmask_d[:, :])
        bmask = const.tile([GPC, P], F32, tag="bmask")
        nc.sync.dma_start(out=bmask, in_=bmask_d[:, :])
        identf8 = const.tile([P, P], F8, tag="identf8")
        nc.sync.dma_start(out=identf8, in_=identf8_d[:, :])
        ones4 = const.tile([P, 2, 1], F8, tag="ones4")
        nc.vector.memset(ones4, S_OV / X_S)
        eps_t = const.tile([GPC, 1], F32, tag="eps")
        nc.vector.memset(eps_t, EPS)

        n_it = reps * NS1
        x_next = xp.tile([P, CCH, L1], BF16, tag="x0")
        nc.sync.dma_start(out=x_next, in_=xs[0].rearrange("(k p) l -> p k l", p=P))
        for i_rep in range(n_it):
            i = i_rep % NS1
            x_sb = x_next
            if i_rep + 1 < n_it:
                x_next = xp.tile([P, CCH, L1], BF16, tag="x0")
                nc.sync.dma_start(
                    out=x_next,
                    in_=xs[(i_rep + 1) % NS1].rearrange("(k p) l -> p k l", p=P))

            # ---- GroupNorm stats (batched over chunks) -> h (fp8) ----
            mv = stp.tile([P, 2, CCH], F32, tag="mv")
            for k in range(CCH):
                xc = x_sb[:, k, :]
                st = stp.tile([P, 2, 6], F32, tag="bnst")
                nc.vector.bn_stats(out=st[:, 0, :], in_=xc[:, 0:512])
                nc.vector.bn_stats(out=st[:, 1, :], in_=xc[:, 512:1024])
                nc.vector.bn_aggr(out=mv[:, :, k], in_=st)
            me = stp.tile([P, 2, CCH], F32, tag="me")
            nc.vector.tensor_copy(out=me[:, 0, :], in_=mv[:, 0, :])
            m2 = stp.tile([P, CCH], F32, tag="m2")
            nc.vector.tensor_mul(out=m2, in0=mv[:, 0, :], in1=mv[:, 0, :])
            nc.vector.tensor_add(out=me[:, 1, :], in0=mv[:, 1, :], in1=m2)
            gs_ps = psS.tile([GPC, 2, CCH], F32, tag="st")
            nc.tensor.matmul(out=gs_ps.rearrange("g a k -> g (a k)"),
                             lhsT=gmask, rhs=me.rearrange("p a k -> p (a k)"),
                             start=True, stop=True)
            gs = stp.tile([GPC, 2, CCH], F32, tag="gs")
            nc.vector.tensor_copy(out=gs, in_=gs_ps)
            var = stp.tile([GPC, CCH], F32, tag="var")
            nc.vector.tensor_mul(out=var, in0=gs[:, 0, :], in1=gs[:, 0, :])
            var2 = stp.tile([GPC, CCH], F32, tag="var2")
            nc.vector.tensor_sub(out=var2, in0=gs[:, 1, :], in1=var)
            # rstd = exp(-0.5*ln(var+eps)) -- stays on the ln/exp table
            lnv = stp.tile([GPC, CCH], F32, tag="lnv")
            nc.scalar.activation(out=lnv, in_=var2, func=AF.Ln, bias=eps_t)
            ab = stp.tile([GPC, 2, CCH], F32, tag="ab")
            nc.scalar.activation(out=ab[:, 0, :], in_=lnv, func=AF.Exp,
                                 scale=-0.5)
            nc.vector.scalar_tensor_tensor(
                out=ab[:, 1, :], in0=gs[:, 0, :], scalar=-1.0, in1=ab[:, 0, :],
                op0=A.mult, op1=A.mult,
            )
            abc_ps = psS.tile([P, 2, CCH], F32, tag="st")
            nc.tensor.matmul(out=abc_ps.rearrange("p a k -> p (a k)"),
                             lhsT=bmask, rhs=ab.rearrange("g a k -> g (a k)"),
                             start=True, stop=True)
            abc = stp.tile([P, 2, CCH], F32, tag="abc")
            nc.vector.tensor_copy(out=abc, in_=abc_ps)
            h_sb = hp.tile([P, CCH, L1], F8, tag="h")
            for k in range(CCH):
                nc.gpsimd.tensor_scalar(
                    out=h_sb[:, k, :], in0=x_sb[:, k, :],
                    scalar1=abc[:, 0, k:k + 1], scalar2=abc[:, 1, k:k + 1],
                    op0=A.mult, op1=A.add,
                )

            # ---- z = M h + bz (c-major, fp8 DoubleRow) ----
            z_sb = zp.tile([P, CCH, L1], F8, tag="z")
            for m in range(CCH):
                ps = psA.tile([P, L1], F32, tag="mm")
                for kk in (0, 2):
                    for nb in range(2):
                        nc.tensor.matmul(
                            out=ps[:, nb * 512:(nb + 1) * 512],
                            lhsT=w_sb["wm"][:, kk:kk + 2, m * P:(m + 1) * P],
                            rhs=h_sb[:, kk:kk + 2, nb * 512:(nb + 1) * 512],
                            start=(kk == 0), stop=(kk == 2),
                            perf_mode=DR,
                        )
                nc.vector.tensor_scalar_add(
                    out=z_sb[:, m, :], in0=ps, scalar1=b_sb["bz"][:, m:m + 1])

            # ---- u^T = (Wo Wv) h, positions on partitions ----
            uT_sb = up.tile([P, LCH1, C], F8, tag="u")
            for m in range(LCH1):
                ps = psB.tile([P, C], F32, tag="u")
                for kk in (0, 2):
                    nc.tensor.matmul(
                        out=ps,
                        lhsT=h_sb[:, kk:kk + 2, m * P:(m + 1) * P],
                        rhs=w_sb["wu"][:, kk:kk + 2, :],
                        start=(kk == 0), stop=(kk == 2),
                        perf_mode=DR,
                    )
                eng = nc.scalar if (m % 2 == 0) else nc.vector
                if eng is nc.scalar:
                    eng.activation(out=uT_sb[:, m, :], in_=ps, func=AF.Copy)
                else:
                    eng.tensor_copy(out=uT_sb[:, m, :], in_=ps)

            # ---- S^T = h^T z per key chunk, exp -> p^T (fp8, direct) ----
            pt_sb = ptp.tile([P, LCH1, L1], F8, tag="ptv")
            for m in range(LCH1):
                ps_s = psA.tile([P, L1], F32, tag="mm")
                for kk in (0, 2):
                    for nb in range(2):
                        nc.tensor.matmul(
                            out=ps_s[:, nb * 512:(nb + 1) * 512],
                            lhsT=h_sb[:, kk:kk + 2, m * P:(m + 1) * P],
                            rhs=z_sb[:, kk:kk + 2, nb * 512:(nb + 1) * 512],
                            start=(kk == 0), stop=(kk == 2),
                            perf_mode=DR,
                        )
                nc.scalar.activation(out=pt_sb[:, m, :], in_=ps_s,
                                     func=AF.Exp, scale=SC_EXP)

            # ---- rowsums (per q partition-chunk) + r^T = p^T' U ----
            rs_ps = psS.tile([P, LCH1], F32, tag="rs")
            rT_sb = rp.tile([P, LCH1, C], F8, tag="rT")
            for qc in range(LCH1):
                for j in range(4):
                    nc.tensor.matmul(
                        out=rs_ps[:, qc:qc + 1],
                        lhsT=pt_sb[:, 2 * j:2 * j + 2, qc * P:(qc + 1) * P],
                        rhs=ones4,
                        start=(j == 0), stop=(j == 3), perf_mode=DR,
                    )
            rc_all = stp.tile([P, LCH1], F32, tag="rca")
            nc.vector.reciprocal(out=rc_all, in_=rs_ps)
            for qc in range(LCH1):
                ps_r = psB.tile([P, C], F32, tag="u")
                for j in range(4):
                    nc.tensor.matmul(
                        out=ps_r,
                        lhsT=pt_sb[:, 2 * j:2 * j + 2, qc * P:(qc + 1) * P],
                        rhs=uT_sb[:, 2 * j:2 * j + 2, :],
                        start=(j == 0), stop=(j == 3), perf_mode=DR,
                    )
                nc.scalar.activation(out=rT_sb[:, qc, :], in_=ps_r,
                                     func=AF.Copy, scale=rc_all[:, qc:qc + 1])

            # ---- transpose r^T -> r, add bias + residual -> ys ----
            for m in range(CCH):
                ps_y = psA.tile([P, L1], F32, tag="mm")
                for qc in range(LCH1):
                    nc.tensor.matmul(
                        out=ps_y[:, qc * P:(qc + 1) * P],
                        lhsT=rT_sb[:, qc, m * P:(m + 1) * P], rhs=identf8,
                        start=True, stop=True,
                    )
                y_sb = yp.tile([P, L1], F32, tag="y")
                nc.vector.scalar_tensor_tensor(
                    out=y_sb, in0=ps_y, scalar=b_sb["bo"][:, m:m + 1],
                    in1=x_sb[:, m, :], op0=A.add, op1=A.add,
                )
                nc.sync.dma_start(out=ys[i, m * P:(m + 1) * P, :], in_=y_sb)
    return nc


# ---------------------------------------------------------------- phase 2
def build_temporal(reps=1):
    nc = bass.Bass()
    xt = nc.dram_tensor("xt", [C, NS2 * NT2], BF16, kind="ExternalInput")
    yt = nc.dram_tensor("yt", [C, NS2 * NT2], F32, kind="ExternalOutput")
    wd = {
        n: nc.dram_tensor(n, [C, C], F8, kind="ExternalInput")
        for n in ("wm", "wv", "wo")
    }
    bd = {
        n: nc.dram_tensor(n, [P, CCH], F32, kind="ExternalInput")
        for n in ("bz", "bo")
    }
    gmask_d = nc.dram_tensor("gmask", [P, GPC], BF16, kind="ExternalInput")
    bmask_d = nc.dram_tensor("bmask", [GPC, P], BF16, kind="ExternalInput")
    ident_d = nc.dram_tensor("ident", [P, P], BF16, kind="ExternalInput")
    blkmask_d = nc.dram_tensor("blkmask", [P, P], F32, kind="ExternalInput")
    A = _op()
    NN = HALF  # samples per half

    with tile.TileContext(nc) as tc, ExitStack() as ctx:
        const = ctx.enter_context(tc.tile_pool(name="const", bufs=1))
        stp = ctx.enter_context(tc.tile_pool(name="stats", bufs=2))
        xp = ctx.enter_context(tc.tile_pool(name="x", bufs=2))
        sqp = ctx.enter_context(tc.tile_pool(name="sq", bufs=2))
        trp = ctx.enter_context(tc.tile_pool(name="tr", bufs=2))
        tmpp = ctx.enter_context(tc.tile_pool(name="tmp", bufs=2))
        hp = ctx.enter_context(tc.tile_pool(name="h", bufs=3))
        zp = ctx.enter_context(tc.tile_pool(name="z", bufs=3))
        vp = ctx.enter_context(tc.tile_pool(name="v", bufs=3))
        pp = ctx.enter_context(tc.tile_pool(name="pm", bufs=3))
        yp = ctx.enter_context(tc.tile_pool(name="y", bufs=3))
        psA = ctx.enter_context(tc.tile_pool(name="psA", bufs=3, space="PSUM"))
        psB = ctx.enter_context(tc.tile_pool(name="psB", bufs=2, space="PSUM"))
        psT = ctx.enter_context(tc.tile_pool(name="psT", bufs=2, space="PSUM"))

        w_sb = {}
        for n in wd:
            t = const.tile([P, CCH, C], F8, tag=n)
            nc.sync.dma_start(out=t, in_=wd[n].rearrange("(k p) o -> p k o", p=P))
            w_sb[n] = t
        b_sb = {}
        for n in bd:
            t = const.tile([P, CCH], F32, tag=n)
            nc.sync.dma_start(out=t, in_=bd[n][:, :])
            b_sb[n] = t
        gmask = const.tile([P, GPC], BF16, tag="gmask")
        nc.sync.dma_start(out=gmask, in_=gmask_d[:, :])
        bmask = const.tile([GPC, P], BF16, tag="bmask")
        nc.sync.dma_start(out=bmask, in_=bmask_d[:, :])
        ident = const.tile([P, P], BF16, tag="ident")
        nc.sync.dma_start(out=ident, in_=ident_d[:, :])
        ident256 = const.tile([P, P], BF16, tag="ident256")
        nc.vector.tensor_scalar_mul(out=ident256, in0=ident, scalar1=DG_S)
        blkmask = const.tile([P, P], F32, tag="blkmask")
        nc.sync.dma_start(out=blkmask, in_=blkmask_d[:, :])
        blkbf = const.tile([P, P], BF16, tag="blkbf")
        nc.vector.tensor_copy(out=blkbf, in_=blkmask)
        eps_t = const.tile([GPC, 1], F32, tag="eps")
        nc.vector.memset(eps_t, EPS)

        xr = xt.rearrange("(k p) f -> p k f", p=P)
        yr = yt.rearrange("(k p) f -> p k f", p=P)

        n_it = reps * 4
        x_next = xp.tile([P, CCH, F2], BF16, tag="x0")
        nc.sync.dma_start(out=x_next, in_=xr[:, :, 0:F2])
        for ih_rep in range(n_it):
            ih = ih_rep % 4
            f0 = ih * F2
            x_sb = x_next
            if ih_rep + 1 < n_it:
                f1 = ((ih_rep + 1) % 4) * F2
                x_next = xp.tile([P, CCH, F2], BF16, tag="x0")
                nc.sync.dma_start(out=x_next, in_=xr[:, :, f1:f1 + F2])

            # ---- GroupNorm stats via halving trees ----
            sq = sqp.tile([P, CCH, F2], BF16, tag="sq")
            nc.gpsimd.tensor_mul(
                out=sq.rearrange("p k f -> p (k f)"),
                in0=x_sb.rearrange("p k f -> p (k f)"),
                in1=x_sb.rearrange("p k f -> p (k f)"))
            me_bf = stp.tile([P, 2, CCH, NN], BF16, tag="mebf")
            with nc.allow_low_precision("GN stats tree in bf16"):
                for src_i, src in ((0, x_sb), (1, sq)):
                    v16 = src.rearrange("p k (n t) -> p (k n) t", t=NT2)
                    t8 = trp.tile([P, CCH * NN, 8], BF16, tag="t8")
                    nc.gpsimd.tensor_tensor(
                        out=t8, in0=v16[:, :, 0:8], in1=v16[:, :, 8:16],
                        op=A.add)
                    t4 = trp.tile([P, CCH * NN, 4], BF16, tag="t4")
                    nc.vector.tensor_tensor(
                        out=t4, in0=t8[:, :, 0:4], in1=t8[:, :, 4:8],
                        op=A.add)
                    t2 = trp.tile([P, CCH * NN, 2], BF16, tag="t2")
                    nc.vector.tensor_tensor(
                        out=t2, in0=t4[:, :, 0:2], in1=t4[:, :, 2:4],
                        op=A.add)
                    nc.vector.reduce_sum(
                        out=me_bf[:, src_i].rearrange("p k n -> p (k n)"),
                        in_=t2, axis=AX)
            gs_ps = psT.tile([GPC, 2, CCH, NN], F32, tag="st")
            for hb in range((2 * CCH * NN) // 512):
                nc.tensor.matmul(
                    out=gs_ps.rearrange("g a k n -> g (a k n)")[:, hb * 512:(hb + 1) * 512],
                    lhsT=gmask,
                    rhs=me_bf.rearrange("p a k n -> p (a k n)")[:, hb * 512:(hb + 1) * 512],
                    start=True, stop=True,
                )
            gs = gs_ps
            var = stp.tile([GPC, CCH, NN], F32, tag="var2a")
            nc.scalar.activation(
                out=var.rearrange("g k n -> g (k n)"),
                in_=gs[:, 0, :, :].rearrange("g k n -> g (k n)"),
                func=AF.Square)
            var2 = stp.tile([GPC, CCH, NN], F32, tag="var2b")
            nc.vector.tensor_sub(out=var2, in0=gs[:, 1, :, :], in1=var)
            lnv = stp.tile([GPC, CCH, NN], F32, tag="lnv")
            nc.scalar.activation(
                out=lnv, in_=var2.rearrange("g k n -> g (k n)"),
                func=AF.Ln, bias=eps_t)
            ab = stp.tile([GPC, 2, CCH, NN], BF16, tag="ab2")
            nc.scalar.activation(
                out=ab[:, 0, :, :], in_=lnv, func=AF.Exp, scale=-0.5)
            nc.vector.scalar_tensor_tensor(
                out=ab[:, 1, :, :], in0=gs[:, 0, :, :], scalar=-1.0,
                in1=ab[:, 0, :, :], op0=A.mult, op1=A.mult,
            )
            abc_ps = psT.tile([P, 2, CCH, NN], F32, tag="st")
            for hb in range((2 * CCH * NN) // 512):
                nc.tensor.matmul(
                    out=abc_ps.rearrange("p a k n -> p (a k n)")[:, hb * 512:(hb + 1) * 512],
                    lhsT=bmask,
                    rhs=ab.rearrange("g a k n -> g (a k n)")[:, hb * 512:(hb + 1) * 512],
                    start=True, stop=True,
                )
            abc = stp.tile([P, 2, CCH, NN], BF16, tag="abc2")
            nc.vector.tensor_copy(out=abc, in_=abc_ps)

            # ---- GN apply -> h (fp8) ----
            h_sb = hp.tile([P, CCH, F2], F8, tag="h")
            for k in range(CCH):
                xc3 = x_sb[:, k, :].rearrange("p (n t) -> p n t", t=NT2)
                tmp = tmpp.tile([P, F2], BF16, tag="tmp")
                nc.vector.tensor_tensor(
                    out=tmp.rearrange("p (n t) -> p n t", t=NT2),
                    in0=xc3, in1=_bcast_inner(abc[:, 0, k, :], NT2), op=A.mult,
                )
                nc.gpsimd.tensor_tensor(
                    out=h_sb[:, k, :].rearrange("p (n t) -> p n t", t=NT2),
                    in0=tmp.rearrange("p (n t) -> p n t", t=NT2),
                    in1=_bcast_inner(abc[:, 1, k, :], NT2), op=A.add,
                )

            # ---- z = M h + bz (fp8 DoubleRow) ----
            z_sb = zp.tile([P, CCH, F2], F8, tag="z")
            for m in range(CCH):
                for nb in range(F2 // 512):
                    ps = psA.tile([P, 512], F32, tag="mm")
                    for kk in (0, 2):
                        nc.tensor.matmul(
                            out=ps,
                            lhsT=w_sb["wm"][:, kk:kk + 2, m * P:(m + 1) * P],
                            rhs=h_sb[:, kk:kk + 2, nb * 512:(nb + 1) * 512],
                            start=(kk == 0), stop=(kk == 2),
                            perf_mode=DR,
                        )
                    nc.scalar.activation(
                        out=z_sb[:, m, nb * 512:(nb + 1) * 512], in_=ps,
                        func=AF.Identity, bias=b_sb["bz"][:, m:m + 1])

            # ---- v^T (fp8 DoubleRow), 2 groups per psum tile ----
            vT_sb = vp.tile([P, NGRP, C], F8, tag="v")
            for m in range(NGRP):
                ps = psA.tile([P, 512], F32, tag="mm")
                for kk in (0, 2):
                    nc.tensor.matmul(
                        out=ps,
                        lhsT=h_sb[:, kk:kk + 2, m * P:(m + 1) * P],
                        rhs=w_sb["wv"][:, kk:kk + 2, :],
                        start=(kk == 0), stop=(kk == 2),
                        perf_mode=DR,
                    )
                nc.scalar.activation(
                    out=vT_sb[:, m, :], in_=ps, func=AF.Copy)

            # ---- attention, GB-group batches ----
            o_sb = hp.tile([P, CCH, F2], F8, tag="h")
            for bat in range(NGRP // GB):
                g0 = bat * GB
                ps_s = psB.tile([P, GB * P], F32, tag="sps")
                for g in range(GB):
                    c0 = (g0 + g) * P
                    nc.tensor.matmul(
                        out=ps_s[:, g * P:(g + 1) * P],
                        lhsT=ident, rhs=blkbf, start=True, stop=False,
                    )
                    for kk in (0, 2):
                        nc.tensor.matmul(
                            out=ps_s[:, g * P:(g + 1) * P],
                            lhsT=z_sb[:, kk:kk + 2, c0:c0 + P],
                            rhs=h_sb[:, kk:kk + 2, c0:c0 + P],
                            start=False, stop=(kk == 2),
                            perf_mode=DR,
                        )
                p_sb = pp.tile([P, GB * P], BF16, tag="pv")
                nc.scalar.activation(out=p_sb, in_=ps_s, func=AF.Exp,
                                     scale=SC_EXP)
                rs = stp.tile([P, GB], F32, tag="rs")
                nc.vector.reduce_sum(
                    out=rs, in_=p_sb.rearrange("p (g q) -> p g q", q=P), axis=AX)
                rc = stp.tile([P, GB], F32, tag="rc")
                nc.vector.reciprocal(out=rc, in_=rs)
                dg = stp.tile([P, GB, P], BF16, tag="dg")
                nc.vector.tensor_tensor(
                    out=dg, in0=_bcast_inner(rc, P),
                    in1=_bcast_outer(ident256, GB), op=A.mult,
                )
                ps_t = psB.tile([P, GB * P], F32, tag="sps")
                for g in range(GB):
                    nc.tensor.matmul(
                        out=ps_t[:, g * P:(g + 1) * P],
                        lhsT=p_sb[:, g * P:(g + 1) * P], rhs=dg[:, g, :],
                        start=True, stop=True,
                    )
                pt_sb = pp.tile([P, GB * P], F8, tag="ptv")
                nc.vector.tensor_copy(out=pt_sb, in_=ps_t)
                for m in range(CCH):
                    ps_o = psA.tile([P, GB * P], F32, tag="mm")
                    for g in range(GB):
                        nc.tensor.matmul(
                            out=ps_o[:, g * P:(g + 1) * P],
                            lhsT=vT_sb[:, g0 + g, m * P:(m + 1) * P],
                            rhs=pt_sb[:, g * P:(g + 1) * P],
                            start=True, stop=True,
                        )
                    nc.scalar.activation(
                        out=o_sb[:, m, g0 * P:(g0 + GB) * P], in_=ps_o,
                        func=AF.Copy, scale=1.0 / (S_W * DG_S))

                # ---- r = Wo O + bo'' + x'' for this batch's columns ----
                for m in range(CCH):
                    ps_r = psA.tile([P, GB * P], F32, tag="mm")
                    for kk in (0, 2):
                        nc.tensor.matmul(
                            out=ps_r,
                            lhsT=w_sb["wo"][:, kk:kk + 2, m * P:(m + 1) * P],
                            rhs=o_sb[:, kk:kk + 2,
                                     g0 * P:(g0 + GB) * P],
                            start=(kk == 0), stop=(kk == 2),
                            perf_mode=DR,
                        )
                    y_sb = yp.tile([P, GB * P], F32, tag="y")
                    nc.vector.scalar_tensor_tensor(
                        out=y_sb, in0=ps_r, scalar=b_sb["bo"][:, m:m + 1],
                        in1=x_sb[:, m, g0 * P:(g0 + GB) * P],
                        op0=A.add, op1=A.add,
                    )
                    nc.sync.dma_start(
                        out=yr[:, m, f0 + g0 * P:f0 + (g0 + GB) * P], in_=y_sb)
    return nc


# ---------------------------------------------------------------- host side
F8NP = mybir.dt.np(mybir.dt.float8e4)


def _gn_fold(w, b, gamma, beta):
    """Fold GN affine into conv: W @ (hn*gamma+beta) + b."""
    w = np.asarray(w, np.float32)
    b = np.asarray(b, np.float32)
    w_eff = w * np.asarray(gamma, np.float32)[None, :]
    b_eff = b + w @ np.asarray(beta, np.float32)
    return w_eff, b_eff


def _to_f8(w, scale):
    return np.clip(np.ascontiguousarray(w) * scale, -240., 240.).astype(F8NP)


def _to_bias(b):
    return np.ascontiguousarray(np.asarray(b, np.float32).reshape(CCH, P).T)


def _consts():
    gmask1 = np.zeros((P, GPC), np.float32)
    for p in range(P):
        gmask1[p, p // GS] = 1.0 / (GS * 1)  # spatial: /16 (channel avg of means)
    gmask2 = np.zeros((P, GPC), np.float32)
    for p in range(P):
        gmask2[p, p // GS] = 1.0 / (GS * NT2)  # temporal: /256 (full group sum)
    bmask = np.zeros((GPC, P), np.float32)
    for p in range(P):
        bmask[p // GS, p] = 1.0
    ident = np.eye(P).astype(ml_dtypes.bfloat16)
    identf8 = np.eye(P).astype(F8NP)
    blk = np.full((P, P), -1e9, np.float32)
    for n in range(P // NT2):
        blk[n * NT2:(n + 1) * NT2, n * NT2:(n + 1) * NT2] = 0.0
    return gmask1, gmask2, bmask, ident, identf8, blk


_CACHE = {}


def _phase_weights(wq, bq, wk, bk, wv, bv, wo, bo, gamma, beta, phase):
    wq_eff, bq_eff = _gn_fold(wq, bq, gamma, beta)
    wk_eff, _ = _gn_fold(wk, bk, gamma, beta)   # bk cancels in softmax
    wv_eff, bv_eff = _gn_fold(wv, bv, gamma, beta)
    wo = np.asarray(wo, np.float32)
    bo = np.asarray(bo, np.float32)
    # fused logit matrix: z = M h + bz,  logits = h^T z
    m_lhsT = wq_eff.T @ wk_eff          # (c_in, c_out)
    bz = wk_eff.T @ bq_eff
    bo_eff = bo + wo @ bv_eff
    out = {
        "wm": _to_f8(m_lhsT, S_M),
        "bz": _to_bias(bz * S_M),
        "bo": _to_bias(bo_eff * X_S),
    }
    if phase == 1:
        # fused output-projection: r = (Wo Wv) h P_hat^T
        out["wu"] = _to_f8(wv_eff.T @ wo.T, S_OV)
    else:
        out["wv"] = _to_f8(wv_eff.T, S_W)
        out["wo"] = _to_f8(wo.T, S_W)
    return out


def kernel(**inputs):
    x = np.asarray(inputs["x"], np.float32)
    gmask1, gmask2, bmask, ident, identf8, blk = _consts()

    w1 = _phase_weights(inputs["wq_s"], inputs["bq_s"], inputs["wk_s"],
                        inputs["bk_s"], inputs["wv_s"], inputs["bv_s"],
                        inputs["wo_s"], inputs["bo_s"],
                        inputs["gamma_s"], inputs["beta_s"], 1)
    w2 = _phase_weights(inputs["wq_t"], inputs["bq_t"], inputs["wk_t"],
                        inputs["bk_t"], inputs["wv_t"], inputs["bv_t"],
                        inputs["wo_t"], inputs["bo_t"],
                        inputs["gamma_t"], inputs["beta_t"], 2)

    if "nc1" not in _CACHE:
        _CACHE["nc1"] = _split_waits(build_spatial())
        _CACHE["nc2"] = _split_waits(build_temporal())
    nc1, nc2 = _CACHE["nc1"], _CACHE["nc2"]

    # ---- phase 1: spatial over (b t); input pre-scaled by 32 ----
    xs = np.ascontiguousarray(
        x.transpose(0, 2, 1, 3, 4).reshape(B * T, C, L1)
    ) * X_S
    common1 = dict(gmask=gmask1, bmask=bmask, identf8=identf8, **w1)
    in_maps1 = [
        dict(xs=np.ascontiguousarray(xs[i * NS1:(i + 1) * NS1])
             .astype(ml_dtypes.bfloat16), **common1)
        for i in range(N_CORES)
    ]
    _CACHE["in_maps1"] = in_maps1
    r1 = run_bass_kernel_spmd(nc1, in_maps1, core_ids=list(range(N_CORES)),
                              **_CACHE.get("run_kwargs", {}))
    ys = np.concatenate([r1.results[i]["ys"] for i in range(N_CORES)], axis=0)
    _CACHE["last_r1"] = r1

    # ---- phase 2: temporal over (b h w); ys already carries the 32x ----
    x2 = ys.reshape(B, T, C, H, W).transpose(0, 3, 4, 2, 1)  # (b,h,w,c,t)
    x2 = x2.reshape(B * H * W, C, NT2)
    common2 = dict(gmask=gmask2.astype(ml_dtypes.bfloat16),
                   bmask=bmask.astype(ml_dtypes.bfloat16),
                   ident=ident, blkmask=blk, **w2)
    in_maps2 = []
    for i in range(N_CORES):
        shard = x2[i * NS2:(i + 1) * NS2]          # (256, 512, 16)
        xt = np.ascontiguousarray(shard.transpose(1, 0, 2)).reshape(
            C, NS2 * NT2).astype(ml_dtypes.bfloat16)
        in_maps2.append(dict(xt=xt, **common2))
    _CACHE["in_maps2"] = in_maps2
    r2 = run_bass_kernel_spmd(nc2, in_maps2, core_ids=list(range(N_CORES)),
                              **_CACHE.get("run_kwargs", {}))
    _CACHE["last_r2"] = r2

    out = np.empty((B * H * W, C, NT2), np.float32)
    for i in range(N_CORES):
        yt = r2.results[i]["yt"].reshape(C, NS2, NT2)
        out[i * NS2:(i + 1) * NS2] = yt.transpose(1, 0, 2)
    out = out.reshape(B, H, W, C, NT2).transpose(0, 3, 4, 1, 2) / X_S
    return np.ascontiguousarray(out)


# revision 25
# speedup vs baseline: 2.6287x; 1.2816x over previous
"""AttnBlockST Trainium2 kernel (fp8 DoubleRow, fused-projection version).

Two SPMD phases on 8 NeuronCores:
  phase 1 (spatial): data-parallel over b*t (32 samples -> 4/core),
    attention over hw=1024 within each (bt, c, hw) sample.
  phase 2 (temporal): data-parallel over b*h*w (2048 -> 256/core),
    attention over t=16, 8 samples packed per 128-partition group with a
    block-diagonal logit mask.

Matmuls run in fp8e4 with DoubleRow perf mode (256-wide contraction,
0.5 cyc/row).  Host-side algebra shrinks the device work:
  * logits = (Wq h + bq)^T (Wk h)  ==  h^T (M h + bz)  with
    M = Wq_eff^T Wk_eff (one projection z instead of q AND k; bk cancels
    in the softmax over keys).
  * phase 1 also fuses Wo (Wv h) = (Wo Wv) h = U h, so the attention
    output projection is applied to V up front and the O stage vanishes.
  * bv is folded into bo (bo_eff = bo + Wo bv).
  * inputs are pre-scaled by 32 (x'' = 32 x, bf16) and outputs carry the
    same 32x factor, so the final residual+bias is one
    scalar_tensor_tensor; the host divides the final output by 32.
GroupNorm statistics, softmax and accumulation stay fp32; rstd is
exp(-0.5*ln(var+eps)) so the scalar engine only ever uses the ln/exp
activation table (no table thrashing).
"""

import numpy as np
import ml_dtypes
from contextlib import ExitStack

import concourse.bass as bass
import concourse.mybir as mybir
import concourse.tile as tile
from concourse.bass_utils import run_bass_kernel_spmd

# ---- walrus workaround: split multi-wait final drain ----
from concourse.vector_clock import ScopedClock
from concourse.tile import TileContext


def _patched_drain_and_barrier(self, tick_clock, wait_clock):
    nc = self.nc
    drain_inst = nc.sync.drain()
    wait_clock.add_sem_waits(
        drain_inst.ins, ScopedClock({None: tick_clock.global_clock})
    )
    si = drain_inst.ins.sync_info
    if si is not None and len(si.on_wait) > 1:
        waits = list(si.on_wait)
        drain_inst.ins.sync_info = mybir.SyncInfo(
            on_wait=waits[:1], on_update=list(si.on_update)
        )
        for w in waits[1:]:
            n = nc.sync.nop(nofuse=True, hint="drain_wait_split")
            n.ins.sync_info = mybir.SyncInfo(on_wait=[w], on_update=[])
    nc.all_engine_barrier()
    assert self.sems is not None
    popped = nc._tile_sem_poison_stack.pop()
    assert popped is self._sem_poison
    nc.clear_and_free_semaphores(list(self.sems.allocated().values()))
    nc.all_engine_barrier()


TileContext._drain_and_barrier = _patched_drain_and_barrier

# ---- problem constants (hardcoded per spec) ----
B, C, T, H, W = 2, 512, 16, 32, 32
GROUPS = 32
EPS = 1e-6
N_CORES = 8
P = 128
CCH = C // P          # 4 channel chunks
GPC = GROUPS // CCH   # 8 groups per 128-channel chunk
GS = C // GROUPS      # 16 channels per group

L1 = H * W            # 1024 spatial positions
NS1 = (B * T) // N_CORES   # 4 samples per core, phase 1
LCH1 = L1 // P        # 8 position chunks

NT2 = 16              # temporal length
NS2 = (B * H * W) // N_CORES  # 256 samples per core, phase 2
HALF = NS2 // 4       # process in quarters of 64 samples
F2 = HALF * NT2       # 1024 free columns per quarter
NGRP = F2 // P        # 8 groups of 8 samples per quarter
GB = 4                # groups per attention sub-batch

X_S = 32.0            # input/output carry scale (x'' = 32 x)
S_W = 32.0            # fp8 weight scale for wv/wo (phase 2)
S_M = 128.0           # fp8 scale for the fused M = Wq^T Wk matrix
S_OV = 128.0          # fp8 scale for the fused U = Wo Wv matrix (phase 1)
DG_S = 256.0          # diag (1/rowsum) scale into fp8 range (phase 2)
SC_EXP = float(C) ** -0.5 / S_M   # logit scale applied inside exp

F32 = mybir.dt.float32
BF16 = mybir.dt.bfloat16
F8 = mybir.dt.float8e4
AX = mybir.AxisListType.X
AF = mybir.ActivationFunctionType
DR = mybir.MatmulPerfMode.DoubleRow


def _op():
    from concourse.alu_op_type import AluOpType
    return AluOpType


def _bcast_inner(ap, n):
    """View (P, F) access pattern as (P, F, n) with stride-0 inner dim."""
    return bass.AP(tensor=ap.tensor, offset=ap.offset, ap=list(ap.ap) + [[0, n]])


def _bcast_outer(ap, n):
    """View (P, F) access pattern as (P, n, F) with stride-0 middle dim."""
    a = list(ap.ap)
    return bass.AP(tensor=ap.tensor, offset=ap.offset,
                   ap=[a[0], [0, n]] + a[1:])


def _split_waits(nc, limit=1):
    """This walrus build rejects >1 sem wait on every ISA template tested
    (LDWEIGHTS, CTRL, ACT, DVE TensorScalar); hoist extra waits onto
    same-engine NoOps placed just before."""
    ctr = [0]
    for f in nc.m.functions:
        for b in f.blocks:
            new = []
            for ins in b.instructions:
                si = getattr(ins, "sync_info", None)
                waits = list(si.on_wait) if si is not None and si.on_wait else []
                lim = limit
                if len(waits) > lim:
                    for w in waits[lim:]:
                        ctr[0] += 1
                        new.append(mybir.InstNoOp(
                            name=f"wsplit-{ctr[0]}",
                            sync_info=mybir.SyncInfo(on_wait=[w], on_update=[]),
                            bass_nofuse=True,
                            engine=ins.engine,
                        ))
                    ins.sync_info = mybir.SyncInfo(
                        on_wait=waits[:lim], on_update=list(si.on_update)
                    )
                new.append(ins)
            b.instructions = new
    return nc


# ---------------------------------------------------------------- phase 1
def build_spatial(reps=1):
    nc = bass.Bass()
    xs = nc.dram_tensor("xs", [NS1, C, L1], BF16, kind="ExternalInput")
    ys = nc.dram_tensor("ys", [NS1, C, L1], F32, kind="ExternalOutput")
    wd = {
        n: nc.dram_tensor(n, [C, C], F8, kind="ExternalInput")
        for n in ("wm", "wu")
    }
    bd = {
        n: nc.dram_tensor(n, [P, CCH], F32, kind="ExternalInput")
        for n in ("bz", "bo")
    }
    gmask_d = nc.dram_tensor("gmask", [P, GPC], F32, kind="ExternalInput")
    bmask_d = nc.dram_tensor("bmask", [GPC, P], F32, kind="ExternalInput")
    identf8_d = nc.dram_tensor("identf8", [P, P], F8, kind="ExternalInput")
    A = _op()

    with tile.TileContext(nc) as tc, ExitStack() as ctx:
        const = ctx.enter_context(tc.tile_pool(name="const", bufs=1))
        stp = ctx.enter_context(tc.tile_pool(name="stats", bufs=3))
        xp = ctx.enter_context(tc.tile_pool(name="x", bufs=2))
        hp = ctx.enter_context(tc.tile_pool(name="h", bufs=3))
        zp = ctx.enter_context(tc.tile_pool(name="z", bufs=3))
        up = ctx.enter_context(tc.tile_pool(name="u", bufs=3))
        ptp = ctx.enter_context(tc.tile_pool(name="pt", bufs=3))
        rp = ctx.enter_context(tc.tile_pool(name="r", bufs=3))
        yp = ctx.enter_context(tc.tile_pool(name="y", bufs=3))
        psA = ctx.enter_context(tc.tile_pool(name="psA", bufs=2, space="PSUM"))
        psB = ctx.enter_context(tc.tile_pool(name="psB", bufs=2, space="PSUM"))
        psS = ctx.enter_context(tc.tile_pool(name="psS", bufs=1, space="PSUM"))

        w_sb = {}
        for n in wd:
            t = const.tile([P, CCH, C], F8, tag=n)
            nc.sync.dma_start(out=t, in_=wd[n].rearrange("(k p) o -> p k o", p=P))
            w_sb[n] = t
        b_sb = {}
        for n in bd:
            t = const.tile([P, CCH], F32, tag=n)
            nc.sync.dma_start(out=t, in_=bd[n][:, :])
            b_sb[n] = t
        gmask = const.tile([P, GPC], F32, tag="gmask")
        nc.sync.dma_start(out=gmask, in_=gmask_d[:, :])
        bmask = const.tile([GPC, P], F32, tag="bmask")
        nc.sync.dma_start(out=bmask, in_=bmask_d[:, :])
        identf8 = const.tile([P, P], F8, tag="identf8")
        nc.sync.dma_start(out=identf8, in_=identf8_d[:, :])
        ones4 = const.tile([P, 2, 1], F8, tag="ones4")
        nc.vector.memset(ones4, S_OV / X_S)
        eps_t = const.tile([GPC, 1], F32, tag="eps")
        nc.vector.memset(eps_t, EPS)

        n_it = reps * NS1
        x_next = xp.tile([P, CCH, L1], BF16, tag="x0")
        nc.sync.dma_start(out=x_next, in_=xs[0].rearrange("(k p) l -> p k l", p=P))
        for i_rep in range(n_it):
            i = i_rep % NS1
            x_sb = x_next
            if i_rep + 1 < n_it:
                x_next = xp.tile([P, CCH, L1], BF16, tag="x0")
                nc.sync.dma_start(
                    out=x_next,
                    in_=xs[(i_rep + 1) % NS1].rearrange("(k p) l -> p k l", p=P))

            # ---- GroupNorm stats (batched over chunks) -> h (fp8) ----
            mv = stp.tile([P, 2, CCH], F32, tag="mv")
            for k in range(CCH):
                xc = x_sb[:, k, :]
                st = stp.tile([P, 2, 6], F32, tag="bnst")
                nc.vector.bn_stats(out=st[:, 0, :], in_=xc[:, 0:512])
                nc.vector.bn_stats(out=st[:, 1, :], in_=xc[:, 512:1024])
                nc.vector.bn_aggr(out=mv[:, :, k], in_=st)
            me = stp.tile([P, 2, CCH], F32, tag="me")
            nc.vector.tensor_copy(out=me[:, 0, :], in_=mv[:, 0, :])
            m2 = stp.tile([P, CCH], F32, tag="m2")
            nc.vector.tensor_mul(out=m2, in0=mv[:, 0, :], in1=mv[:, 0, :])
            nc.vector.tensor_add(out=me[:, 1, :], in0=mv[:, 1, :], in1=m2)
            gs_ps = psS.tile([GPC, 2, CCH], F32, tag="st")
            nc.tensor.matmul(out=gs_ps.rearrange("g a k -> g (a k)"),
                             lhsT=gmask, rhs=me.rearrange("p a k -> p (a k)"),
                             start=True, stop=True)
            gs = stp.tile([GPC, 2, CCH], F32, tag="gs")
            nc.vector.tensor_copy(out=gs, in_=gs_ps)
            var = stp.tile([GPC, CCH], F32, tag="var")
            nc.vector.tensor_mul(out=var, in0=gs[:, 0, :], in1=gs[:, 0, :])
            var2 = stp.tile([GPC, CCH], F32, tag="var2")
            nc.vector.tensor_sub(out=var2, in0=gs[:, 1, :], in1=var)
            # rstd = exp(-0.5*ln(var+eps)) -- stays on the ln/exp table
            lnv = stp.tile([GPC, CCH], F32, tag="lnv")
            nc.scalar.activation(out=lnv, in_=var2, func=AF.Ln, bias=eps_t)
            ab = stp.tile([GPC, 2, CCH], F32, tag="ab")
            nc.scalar.activation(out=ab[:, 0, :], in_=lnv, func=AF.Exp,
                                 scale=-0.5)
            nc.vector.scalar_tensor_tensor(
                out=ab[:, 1, :], in0=gs[:, 0, :], scalar=-1.0, in1=ab[:, 0, :],
                op0=A.mult, op1=A.mult,
            )
            abc_ps = psS.tile([P, 2, CCH], F32, tag="st")
            nc.tensor.matmul(out=abc_ps.rearrange("p a k -> p (a k)"),
                             lhsT=bmask, rhs=ab.rearrange("g a k -> g (a k)"),
                             start=True, stop=True)
            abc = stp.tile([P, 2, CCH], F32, tag="abc")
            nc.vector.tensor_copy(out=abc, in_=abc_ps)
            h_sb = hp.tile([P, CCH, L1], F8, tag="h")
            for k in range(CCH):
                nc.gpsimd.tensor_scalar(
                    out=h_sb[:, k, :], in0=x_sb[:, k, :],
                    scalar1=abc[:, 0, k:k + 1], scalar2=abc[:, 1, k:k + 1],
                    op0=A.mult, op1=A.add,
                )

            # ---- z = M h + bz (c-major, fp8 DoubleRow) ----
            z_sb = zp.tile([P, CCH, L1], F8, tag="z")
            for m in range(CCH):
                ps = psA.tile([P, L1], F32, tag="mm")
                for kk in (0, 2):
                    for nb in range(2):
                        nc.tensor.matmul(
                            out=ps[:, nb * 512:(nb + 1) * 512],
                            lhsT=w_sb["wm"][:, kk:kk + 2, m * P:(m + 1) * P],
                            rhs=h_sb[:, kk:kk + 2, nb * 512:(nb + 1) * 512],
                            start=(kk == 0), stop=(kk == 2),
                            perf_mode=DR,
                        )
                nc.vector.tensor_scalar_add(
                    out=z_sb[:, m, :], in0=ps, scalar1=b_sb["bz"][:, m:m + 1])

            # ---- u^T = (Wo Wv) h, positions on partitions ----
            uT_sb = up.tile([P, LCH1, C], F8, tag="u")
            for m in range(LCH1):
                ps = psB.tile([P, C], F32, tag="u")
                for kk in (0, 2):
                    nc.tensor.matmul(
                        out=ps,
                        lhsT=h_sb[:, kk:kk + 2, m * P:(m + 1) * P],
                        rhs=w_sb["wu"][:, kk:kk + 2, :],
                        start=(kk == 0), stop=(kk == 2),
                        perf_mode=DR,
                    )
                eng = nc.scalar if (m % 2 == 0) else nc.vector
                if eng is nc.scalar:
                    eng.activation(out=uT_sb[:, m, :], in_=ps, func=AF.Copy)
                else:
                    eng.tensor_copy(out=uT_sb[:, m, :], in_=ps)

            # ---- S^T = h^T z per key chunk, exp -> p^T (fp8, direct) ----
            pt_sb = ptp.tile([P, LCH1, L1], F8, tag="ptv")
            for m in range(LCH1):
                ps_s = psA.tile([P, L1], F32, tag="mm")
                for kk in (0, 2):
                    for nb in range(2):
                        nc.tensor.matmul(
                            out=ps_s[:, nb * 512:(nb + 1) * 512],
                            lhsT=h_sb[:, kk:kk + 2, m * P:(m + 1) * P],
                            rhs=z_sb[:, kk:kk + 2, nb * 512:(nb + 1) * 512],
                            start=(kk == 0), stop=(kk == 2),
                            perf_mode=DR,
                        )
                nc.scalar.activation(out=pt_sb[:, m, :], in_=ps_s,
                                     func=AF.Exp, scale=SC_EXP)

            # ---- rowsums (per q partition-chunk) + r^T = p^T' U ----
            rs_ps = psS.tile([P, LCH1], F32, tag="rs")
            rT_sb = rp.tile([P, LCH1, C], F8, tag="rT")
            for qc in range(LCH1):
                for j in range(4):
                    nc.tensor.matmul(
                        out=rs_ps[:, qc:qc + 1],
                        lhsT=pt_sb[:, 2 * j:2 * j + 2, qc * P:(qc + 1) * P],
                        rhs=ones4,
                        start=(j == 0), stop=(j == 3), perf_mode=DR,
                    )
            rc_all = stp.tile([P, LCH1], F32, tag="rca")
            nc.vector.reciprocal(out=rc_all, in_=rs_ps)
            for qc in range(LCH1):
                ps_r = psB.tile([P, C], F32, tag="u")
                for j in range(4):
                    nc.tensor.matmul(
                        out=ps_r,
                        lhsT=pt_sb[:, 2 * j:2 * j + 2, qc * P:(qc + 1) * P],
                        rhs=uT_sb[:, 2 * j:2 * j + 2, :],
                        start=(j == 0), stop=(j == 3), perf_mode=DR,
                    )
                nc.scalar.activation(out=rT_sb[:, qc, :], in_=ps_r,
                                     func=AF.Copy, scale=rc_all[:, qc:qc + 1])

            # ---- transpose r^T -> r, add bias + residual -> ys ----
            for m in range(CCH):
                ps_y = psA.tile([P, L1], F32, tag="mm")
                for qc in range(LCH1):
                    nc.tensor.matmul(
                        out=ps_y[:, qc * P:(qc + 1) * P],
                        lhsT=rT_sb[:, qc, m * P:(m + 1) * P], rhs=identf8,
                        start=True, stop=True,
                    )
                y_sb = yp.tile([P, L1], F32, tag="y")
                nc.vector.scalar_tensor_tensor(
                    out=y_sb, in0=ps_y, scalar=b_sb["bo"][:, m:m + 1],
                    in1=x_sb[:, m, :], op0=A.add, op1=A.add,
                )
                nc.sync.dma_start(out=ys[i, m * P:(m + 1) * P, :], in_=y_sb)
    return nc


# ---------------------------------------------------------------- phase 2
def build_temporal(reps=1):
    nc = bass.Bass()
    xt = nc.dram_tensor("xt", [C, NS2 * NT2], BF16, kind="ExternalInput")
    yt = nc.dram_tensor("yt", [C, NS2 * NT2], F32, kind="ExternalOutput")
    wd = {
        n: nc.dram_tensor(n, [C, C], F8, kind="ExternalInput")
        for n in ("wm", "wv", "wo")
    }
    bd = {
        n: nc.dram_tensor(n, [P, CCH], F32, kind="ExternalInput")
        for n in ("bz", "bo")
    }
    gmask_d = nc.dram_tensor("gmask", [P, GPC], BF16, kind="ExternalInput")
    bmask_d = nc.dram_tensor("bmask", [GPC, P], BF16, kind="ExternalInput")
    ident_d = nc.dram_tensor("ident", [P, P], BF16, kind="ExternalInput")
    blkmask_d = nc.dram_tensor("blkmask", [P, P], F32, kind="ExternalInput")
    A = _op()
    NN = HALF  # samples per quarter
    NQ = 4
    NB = NGRP // GB  # attention sub-batches per quarter

    with tile.TileContext(nc) as tc, ExitStack() as ctx:
        const = ctx.enter_context(tc.tile_pool(name="const", bufs=1))
        stp = ctx.enter_context(tc.tile_pool(name="stats", bufs=3))
        dgp = ctx.enter_context(tc.tile_pool(name="dgs", bufs=4))
        xp = ctx.enter_context(tc.tile_pool(name="x", bufs=5))
        sqp = ctx.enter_context(tc.tile_pool(name="sq", bufs=2))
        trp = ctx.enter_context(tc.tile_pool(name="tr", bufs=2))
        tmpp = ctx.enter_context(tc.tile_pool(name="tmp", bufs=3))
        hp = ctx.enter_context(tc.tile_pool(name="h", bufs=5))
        op_ = ctx.enter_context(tc.tile_pool(name="o", bufs=3))
        zp = ctx.enter_context(tc.tile_pool(name="z", bufs=5))
        vp = ctx.enter_context(tc.tile_pool(name="v", bufs=5))
        pp = ctx.enter_context(tc.tile_pool(name="pm", bufs=4))
        yp = ctx.enter_context(tc.tile_pool(name="y", bufs=3))
        psA = ctx.enter_context(tc.tile_pool(name="psA", bufs=3, space="PSUM"))
        psB = ctx.enter_context(tc.tile_pool(name="psB", bufs=3, space="PSUM"))
        psT = ctx.enter_context(tc.tile_pool(name="psT", bufs=2, space="PSUM"))

        w_sb = {}
        for n in wd:
            t = const.tile([P, CCH, C], F8, tag=n)
            nc.sync.dma_start(out=t, in_=wd[n].rearrange("(k p) o -> p k o", p=P))
            w_sb[n] = t
        b_sb = {}
        for n in bd:
            t = const.tile([P, CCH], F32, tag=n)
            nc.sync.dma_start(out=t, in_=bd[n][:, :])
            b_sb[n] = t
        gmask = const.tile([P, GPC], BF16, tag="gmask")
        nc.sync.dma_start(out=gmask, in_=gmask_d[:, :])
        bmask = const.tile([GPC, P], BF16, tag="bmask")
        nc.sync.dma_start(out=bmask, in_=bmask_d[:, :])
        ident = const.tile([P, P], BF16, tag="ident")
        nc.sync.dma_start(out=ident, in_=ident_d[:, :])
        ident256 = const.tile([P, P], BF16, tag="ident256")
        nc.vector.tensor_scalar_mul(out=ident256, in0=ident, scalar1=DG_S)
        blkmask = const.tile([P, P], F32, tag="blkmask")
        nc.sync.dma_start(out=blkmask, in_=blkmask_d[:, :])
        blkbf = const.tile([P, P], BF16, tag="blkbf")
        nc.vector.tensor_copy(out=blkbf, in_=blkmask)
        eps_t = const.tile([GPC, 1], F32, tag="eps")
        nc.vector.memset(eps_t, EPS)

        xr = xt.rearrange("(k p) f -> p k f", p=P)
        yr = yt.rearrange("(k p) f -> p k f", p=P)

        for rep in range(reps):
            x_t, abc_t, h_t, z_t, v_t = {}, {}, {}, {}, {}

            # ---- stage A1: loads + GroupNorm stats per quarter ----
            for q in range(NQ):
                f0 = q * F2
                x_sb = xp.tile([P, CCH, F2], BF16, tag="x0")
                nc.sync.dma_start(out=x_sb, in_=xr[:, :, f0:f0 + F2])
                x_t[q] = x_sb
                sq = sqp.tile([P, CCH, F2], BF16, tag="sq")
                nc.gpsimd.tensor_mul(
                    out=sq.rearrange("p k f -> p (k f)"),
                    in0=x_sb.rearrange("p k f -> p (k f)"),
                    in1=x_sb.rearrange("p k f -> p (k f)"))
                me_bf = stp.tile([P, 2, CCH, NN], BF16, tag="mebf")
                with nc.allow_low_precision("GN stats tree in bf16"):
                    for src_i, src in ((0, x_sb), (1, sq)):
                        v16 = src.rearrange("p k (n t) -> p (k n) t", t=NT2)
                        t8 = trp.tile([P, CCH * NN, 8], BF16, tag="t8")
                        nc.gpsimd.tensor_tensor(
                            out=t8, in0=v16[:, :, 0:8], in1=v16[:, :, 8:16],
                            op=A.add)
                        t4 = trp.tile([P, CCH * NN, 4], BF16, tag="t4")
                        nc.vector.tensor_tensor(
                            out=t4, in0=t8[:, :, 0:4], in1=t8[:, :, 4:8],
                            op=A.add)
                        t2 = trp.tile([P, CCH * NN, 2], BF16, tag="t2")
                        nc.vector.tensor_tensor(
                            out=t2, in0=t4[:, :, 0:2], in1=t4[:, :, 2:4],
                            op=A.add)
                        nc.vector.reduce_sum(
                            out=me_bf[:, src_i].rearrange("p k n -> p (k n)"),
                            in_=t2, axis=AX)
                gs_ps = psT.tile([GPC, 2, CCH, NN], F32, tag="st")
                nc.tensor.matmul(
                    out=gs_ps.rearrange("g a k n -> g (a k n)"),
                    lhsT=gmask,
                    rhs=me_bf.rearrange("p a k n -> p (a k n)"),
                    start=True, stop=True,
                )
                var = stp.tile([GPC, CCH, NN], F32, tag="var2a")
                nc.scalar.activation(
                    out=var.rearrange("g k n -> g (k n)"),
                    in_=gs_ps[:, 0, :, :].rearrange("g k n -> g (k n)"),
                    func=AF.Square)
                var2 = stp.tile([GPC, CCH, NN], F32, tag="var2b")
                nc.vector.tensor_sub(out=var2, in0=gs_ps[:, 1, :, :], in1=var)
                lnv = stp.tile([GPC, CCH, NN], F32, tag="lnv")
                nc.scalar.activation(
                    out=lnv, in_=var2.rearrange("g k n -> g (k n)"),
                    func=AF.Ln, bias=eps_t)
                ab = stp.tile([GPC, 2, CCH, NN], BF16, tag="ab2")
                nc.scalar.activation(
                    out=ab[:, 0, :, :], in_=lnv, func=AF.Exp, scale=-0.5)
                nc.vector.scalar_tensor_tensor(
                    out=ab[:, 1, :, :], in0=gs_ps[:, 0, :, :], scalar=-1.0,
                    in1=ab[:, 0, :, :], op0=A.mult, op1=A.mult,
                )
                abc_ps = psT.tile([P, 2, CCH, NN], F32, tag="st")
                nc.tensor.matmul(
                    out=abc_ps.rearrange("p a k n -> p (a k n)"),
                    lhsT=bmask,
                    rhs=ab.rearrange("g a k n -> g (a k n)"),
                    start=True, stop=True,
                )
                abc = stp.tile([P, 2, CCH, NN], BF16, tag="abc2")
                nc.vector.tensor_copy(out=abc, in_=abc_ps)
                abc_t[q] = abc

            # ---- stage A2: GN apply -> h per quarter ----
            for q in range(NQ):
                x_sb, abc = x_t[q], abc_t[q]
                h_sb = hp.tile([P, CCH, F2], F8, tag="h")
                for k in range(CCH):
                    xc3 = x_sb[:, k, :].rearrange("p (n t) -> p n t", t=NT2)
                    tmp = tmpp.tile([P, F2], BF16, tag="tmp")
                    nc.vector.tensor_tensor(
                        out=tmp.rearrange("p (n t) -> p n t", t=NT2),
                        in0=xc3, in1=_bcast_inner(abc[:, 0, k, :], NT2),
                        op=A.mult,
                    )
                    nc.gpsimd.tensor_tensor(
                        out=h_sb[:, k, :].rearrange("p (n t) -> p n t", t=NT2),
                        in0=tmp.rearrange("p (n t) -> p n t", t=NT2),
                        in1=_bcast_inner(abc[:, 1, k, :], NT2), op=A.add,
                    )
                h_t[q] = h_sb

            # ---- stage B: z and v^T projections per quarter ----
            for q in range(NQ):
                h_sb = h_t[q]
                z_sb = zp.tile([P, CCH, F2], F8, tag="z")
                for m in range(CCH):
                    for nb in range(F2 // 512):
                        ps = psA.tile([P, 512], F32, tag="mm")
                        for kk in (0, 2):
                            nc.tensor.matmul(
                                out=ps,
                                lhsT=w_sb["wm"][:, kk:kk + 2, m * P:(m + 1) * P],
                                rhs=h_sb[:, kk:kk + 2, nb * 512:(nb + 1) * 512],
                                start=(kk == 0), stop=(kk == 2),
                                perf_mode=DR,
                            )
                        nc.scalar.activation(
                            out=z_sb[:, m, nb * 512:(nb + 1) * 512], in_=ps,
                            func=AF.Identity, bias=b_sb["bz"][:, m:m + 1])
                z_t[q] = z_sb
            for q in range(NQ):
                h_sb = h_t[q]
                vT_sb = vp.tile([P, NGRP, C], F8, tag="v")
                for m in range(NGRP):
                    ps = psA.tile([P, 512], F32, tag="mm")
                    for kk in (0, 2):
                        nc.tensor.matmul(
                            out=ps,
                            lhsT=h_sb[:, kk:kk + 2, m * P:(m + 1) * P],
                            rhs=w_sb["wv"][:, kk:kk + 2, :],
                            start=(kk == 0), stop=(kk == 2),
                            perf_mode=DR,
                        )
                    nc.scalar.activation(
                        out=vT_sb[:, m, :], in_=ps, func=AF.Copy)
                v_t[q] = vT_sb

            # ---- stage C: attention, software-pipelined over sub-batches ----
            bats = [(q, b) for q in range(NQ) for b in range(NB)]
            o_t = {}
            for q in range(NQ):
                o_sb = op_.tile([P, CCH, F2], F8, tag="o")
                o_t[q] = o_sb
            nbat = len(bats)
            ps_s_t, p_t, dgrc_t, pt_t = {}, {}, {}, {}
            for i in range(nbat + 4):
                if i < nbat:                       # S + mask preload
                    q, b = bats[i]
                    g0 = b * GB
                    ps_s = psB.tile([P, GB * P], F32, tag="sps")
                    for g in range(GB):
                        c0 = (g0 + g) * P
                        nc.tensor.matmul(
                            out=ps_s[:, g * P:(g + 1) * P],
                            lhsT=ident, rhs=blkbf, start=True, stop=False,
                        )
                        for kk in (0, 2):
                            nc.tensor.matmul(
                                out=ps_s[:, g * P:(g + 1) * P],
                                lhsT=z_t[q][:, kk:kk + 2, c0:c0 + P],
                                rhs=h_t[q][:, kk:kk + 2, c0:c0 + P],
                                start=False, stop=(kk == 2),
                                perf_mode=DR,
                            )
                    ps_s_t[i] = ps_s
                if 1 <= i < nbat + 1:              # exp + rowsums + dg
                    j = i - 1
                    ps_s = ps_s_t.pop(j)
                    p_sb = pp.tile([P, GB * P], BF16, tag="pv")
                    nc.scalar.activation(out=p_sb, in_=ps_s, func=AF.Exp,
                                         scale=SC_EXP)
                    rs = dgp.tile([P, GB], F32, tag="rs")
                    nc.vector.reduce_sum(
                        out=rs, in_=p_sb.rearrange("p (g q) -> p g q", q=P),
                        axis=AX)
                    rc = dgp.tile([P, GB], F32, tag="rc")
                    nc.vector.reciprocal(out=rc, in_=rs)
                    dg = dgp.tile([P, GB, P], BF16, tag="dg")
                    nc.vector.tensor_tensor(
                        out=dg, in0=_bcast_inner(rc, P),
                        in1=_bcast_outer(ident256, GB), op=A.mult,
                    )
                    p_t[j] = p_sb
                    dgrc_t[j] = dg
                if 2 <= i < nbat + 2:              # P^T transpose + fp8 copy
                    j = i - 2
                    p_sb, dg = p_t.pop(j), dgrc_t.pop(j)
                    ps_t = psB.tile([P, GB * P], F32, tag="sps")
                    for g in range(GB):
                        nc.tensor.matmul(
                            out=ps_t[:, g * P:(g + 1) * P],
                            lhsT=p_sb[:, g * P:(g + 1) * P], rhs=dg[:, g, :],
                            start=True, stop=True,
                        )
                    pt_sb = pp.tile([P, GB * P], F8, tag="ptv")
                    nc.vector.tensor_copy(out=pt_sb, in_=ps_t)
                    pt_t[j] = pt_sb
                if 3 <= i < nbat + 3:              # O = v P^T
                    j = i - 3
                    q, b = bats[j]
                    g0 = b * GB
                    pt_sb = pt_t.pop(j)
                    for m in range(CCH):
                        ps_o = psA.tile([P, GB * P], F32, tag="mm")
                        for g in range(GB):
                            nc.tensor.matmul(
                                out=ps_o[:, g * P:(g + 1) * P],
                                lhsT=v_t[q][:, g0 + g, m * P:(m + 1) * P],
                                rhs=pt_sb[:, g * P:(g + 1) * P],
                                start=True, stop=True,
                            )
                        nc.scalar.activation(
                            out=o_t[q][:, m, g0 * P:(g0 + GB) * P], in_=ps_o,
                            func=AF.Copy, scale=1.0 / (S_W * DG_S))
                if 4 <= i:                         # r = Wo O + bo'' + x''
                    j = i - 4
                    q, b = bats[j]
                    g0 = b * GB
                    f0 = q * F2
                    for m in range(CCH):
                        ps_r = psA.tile([P, GB * P], F32, tag="mm")
                        for kk in (0, 2):
                            nc.tensor.matmul(
                                out=ps_r,
                                lhsT=w_sb["wo"][:, kk:kk + 2, m * P:(m + 1) * P],
                                rhs=o_t[q][:, kk:kk + 2, g0 * P:(g0 + GB) * P],
                                start=(kk == 0), stop=(kk == 2),
                                perf_mode=DR,
                            )
                        y_sb = yp.tile([P, GB * P], F32, tag="y")
                        nc.vector.scalar_tensor_tensor(
                            out=y_sb, in0=ps_r, scalar=b_sb["bo"][:, m:m + 1],
                            in1=x_t[q][:, m, g0 * P:(g0 + GB) * P],
                            op0=A.add, op1=A.add,
                        )
                        nc.gpsimd.dma_start(
                            out=yr[:, m, f0 + g0 * P:f0 + (g0 + GB) * P],
                            in_=y_sb)
    return nc


# ---------------------------------------------------------------- host side
F8NP = mybir.dt.np(mybir.dt.float8e4)


def _gn_fold(w, b, gamma, beta):
    """Fold GN affine into conv: W @ (hn*gamma+beta) + b."""
    w = np.asarray(w, np.float32)
    b = np.asarray(b, np.float32)
    w_eff = w * np.asarray(gamma, np.float32)[None, :]
    b_eff = b + w @ np.asarray(beta, np.float32)
    return w_eff, b_eff


def _to_f8(w, scale):
    return np.clip(np.ascontiguousarray(w) * scale, -240., 240.).astype(F8NP)


def _to_bias(b):
    return np.ascontiguousarray(np.asarray(b, np.float32).reshape(CCH, P).T)


def _consts():
    gmask1 = np.zeros((P, GPC), np.float32)
    for p in range(P):
        gmask1[p, p // GS] = 1.0 / (GS * 1)  # spatial: /16 (channel avg of means)
    gmask2 = np.zeros((P, GPC), np.float32)
    for p in range(P):
        gmask2[p, p // GS] = 1.0 / (GS * NT2)  # temporal: /256 (full group sum)
    bmask = np.zeros((GPC, P), np.float32)
    for p in range(P):
        bmask[p // GS, p] = 1.0
    ident = np.eye(P).astype(ml_dtypes.bfloat16)
    identf8 = np.eye(P).astype(F8NP)
    blk = np.full((P, P), -1e9, np.float32)
    for n in range(P // NT2):
        blk[n * NT2:(n + 1) * NT2, n * NT2:(n + 1) * NT2] = 0.0
    return gmask1, gmask2, bmask, ident, identf8, blk


_CACHE = {}


def _phase_weights(wq, bq, wk, bk, wv, bv, wo, bo, gamma, beta, phase):
    wq_eff, bq_eff = _gn_fold(wq, bq, gamma, beta)
    wk_eff, _ = _gn_fold(wk, bk, gamma, beta)   # bk cancels in softmax
    wv_eff, bv_eff = _gn_fold(wv, bv, gamma, beta)
    wo = np.asarray(wo, np.float32)
    bo = np.asarray(bo, np.float32)
    # fused logit matrix: z = M h + bz,  logits = h^T z
    m_lhsT = wq_eff.T @ wk_eff          # (c_in, c_out)
    bz = wk_eff.T @ bq_eff
    bo_eff = bo + wo @ bv_eff
    out = {
        "wm": _to_f8(m_lhsT, S_M),
        "bz": _to_bias(bz * S_M),
        "bo": _to_bias(bo_eff * X_S),
    }
    if phase == 1:
        # fused output-projection: r = (Wo Wv) h P_hat^T
        out["wu"] = _to_f8(wv_eff.T @ wo.T, S_OV)
    else:
        out["wv"] = _to_f8(wv_eff.T, S_W)
        out["wo"] = _to_f8(wo.T, S_W)
    return out


def kernel(**inputs):
    x = np.asarray(inputs["x"], np.float32)
    gmask1, gmask2, bmask, ident, identf8, blk = _consts()

    w1 = _phase_weights(inputs["wq_s"], inputs["bq_s"], inputs["wk_s"],
                        inputs["bk_s"], inputs["wv_s"], inputs["bv_s"],
                        inputs["wo_s"], inputs["bo_s"],
                        inputs["gamma_s"], inputs["beta_s"], 1)
    w2 = _phase_weights(inputs["wq_t"], inputs["bq_t"], inputs["wk_t"],
                        inputs["bk_t"], inputs["wv_t"], inputs["bv_t"],
                        inputs["wo_t"], inputs["bo_t"],
                        inputs["gamma_t"], inputs["beta_t"], 2)

    if "nc1" not in _CACHE:
        _CACHE["nc1"] = _split_waits(build_spatial())
        _CACHE["nc2"] = _split_waits(build_temporal())
    nc1, nc2 = _CACHE["nc1"], _CACHE["nc2"]

    # ---- phase 1: spatial over (b t); input pre-scaled by 32 ----
    xs = np.ascontiguousarray(
        x.transpose(0, 2, 1, 3, 4).reshape(B * T, C, L1)
    ) * X_S
    common1 = dict(gmask=gmask1, bmask=bmask, identf8=identf8, **w1)
    in_maps1 = [
        dict(xs=np.ascontiguousarray(xs[i * NS1:(i + 1) * NS1])
             .astype(ml_dtypes.bfloat16), **common1)
        for i in range(N_CORES)
    ]
    _CACHE["in_maps1"] = in_maps1
    r1 = run_bass_kernel_spmd(nc1, in_maps1, core_ids=list(range(N_CORES)),
                              **_CACHE.get("run_kwargs", {}))
    ys = np.concatenate([r1.results[i]["ys"] for i in range(N_CORES)], axis=0)
    _CACHE["last_r1"] = r1

    # ---- phase 2: temporal over (b h w); ys already carries the 32x ----
    x2 = ys.reshape(B, T, C, H, W).transpose(0, 3, 4, 2, 1)  # (b,h,w,c,t)
    x2 = x2.reshape(B * H * W, C, NT2)
    common2 = dict(gmask=gmask2.astype(ml_dtypes.bfloat16),
                   bmask=bmask.astype(ml_dtypes.bfloat16),
                   ident=ident, blkmask=blk, **w2)
    in_maps2 = []
    for i in range(N_CORES):
        shard = x2[i * NS2:(i + 1) * NS2]          # (256, 512, 16)
        xt = np.ascontiguousarray(shard.transpose(1, 0, 2)).reshape(
            C, NS2 * NT2).astype(ml_dtypes.bfloat16)
        in_maps2.append(dict(xt=xt, **common2))
    _CACHE["in_maps2"] = in_maps2
    r2 = run_bass_kernel_spmd(nc2, in_maps2, core_ids=list(range(N_CORES)),
                              **_CACHE.get("run_kwargs", {}))
    _CACHE["last_r2"] = r2

    out = np.empty((B * H * W, C, NT2), np.float32)
    for i in range(N_CORES):
        yt = r2.results[i]["yt"].reshape(C, NS2, NT2)
        out[i * NS2:(i + 1) * NS2] = yt.transpose(1, 0, 2)
    out = out.reshape(B, H, W, C, NT2).transpose(0, 3, 4, 1, 2) / X_S
    return np.ascontiguousarray(out)
